# revision 8
# baseline (speedup 1.0000x reference)
"""Trainium2 Bass kernel for nn_ClassifyModelMOE (conv feature extractor +
top-3-of-5 MoE + softmax head). Data-parallel over batch across 8 cores.

Self-contained: hardcodes all shapes; builds footprint-packed conv1 inputs and
Toeplitz-expanded conv weights on the host; runs one SPMD Bass/Tile program on
cores 0-7 via run_bass_kernel_spmd.

Structure per 512-token chunk:
  - 2 batched DMAs bring in footprint-packed x tiles (tA [100,24,512],
    tB [60,24,512]); conv1 is ONE matmul per output tile (K=95..100).
  - max-pool reads conv1 PSUM pairs directly (DVE), horizontal max on GpSimd,
    relu+bias fused into a DVE tensor_scalar on the pooled data.
  - conv2 blocks are interleaved into the conv1 row loop to keep PE fed.
  - expert-1 weights are SBUF-resident (loaded once).
"""
import os
import sys
import contextlib

sys.path.insert(0, "/opt/trn_rl_repo")

import numpy as np
import ml_dtypes

import concourse.bacc as bacc
import concourse.mybir as mybir
import concourse.tile as tile
from concourse.bass_utils import run_bass_kernel_spmd
from concourse.masks import make_identity

F32 = mybir.dt.float32
BF16 = mybir.dt.bfloat16
AF = mybir.ActivationFunctionType
ALU = mybir.AluOpType
AX = mybir.AxisListType

NCORES = 8
B = 8192
BC = B // NCORES          # tokens per core
NB = 512                  # batch chunk (columns per matmul)
NCH = BC // NB            # chunks per core
NE, TOPK = 5, 3
DH = 128

# conv1 output geometry: 16ch x 24x24; M-layout (per output row y):
#   Mc0: even x = 2j, j=0..7   -> m = j*16 + o        (128)
#   Mc2: odd x = 2j+1, j=0..7  -> m = j*16 + o        (128)
#   Mc1: x=16..23: [even j=8..11 | odd j=8..11]       (64+64)
# conv1 K-packing (footprint tiles):
#   tA part p = r*20 + c  <-> x[y+r, c],    c=0..19  (serves Mc0 and Mc2)
#   tB part p = r*12 + cc <-> x[y+r, 16+cc], cc=0..11 (serves Mc1)
# pooled row tiles: pp0 = j 0..7 (128 parts: j*16+c), pp1 = j 8..11 (64 parts)
# conv2 output (per row y): M = xout*32 + o2:
#   Mc0: xout 0..3 (128), Mc1: xout 4..7 (128),
#   Mc2pair: [y0: xout 8..9 | y1: xout 8..9] (64+64)


def _host_prep(x, conv1_w, conv1_b, conv2_w, conv2_b, gate_w, gate_b,
               e1_w, e1_b, e2_w, e2_b, sm_w, sm_b):
    bf = ml_dtypes.bfloat16
    x = np.asarray(x, np.float32)
    conv1_w = np.asarray(conv1_w, np.float32)
    conv2_w = np.asarray(conv2_w, np.float32)
    gate_w = np.asarray(gate_w, np.float32)
    e1_w = np.asarray(e1_w, np.float32)
    e2_w = np.asarray(e2_w, np.float32)

    xr = x.reshape(B, 28, 28)

    # footprint-packed x for conv1 (host-duplicated, per-core sliced below)
    xa_full = np.empty((100, B, 24), np.float32)
    for r in range(5):
        for c in range(20):
            xa_full[r * 20 + c] = xr[:, r:r + 24, c]
    xb_full = np.empty((60, B, 24), np.float32)
    for r in range(5):
        for cc in range(12):
            xb_full[r * 12 + cc] = xr[:, r:r + 24, 16 + cc]
    xa_full = xa_full.astype(bf)
    xb_full = xb_full.astype(bf)

    percore = []
    for cid in range(NCORES):
        c0 = cid * BC
        xa = xa_full[:, c0:c0 + BC, :].reshape(100, NCH, NB, 24)
        xa = np.ascontiguousarray(xa.transpose(0, 1, 3, 2))   # [100,NCH,24,NB]
        xb = xb_full[:, c0:c0 + BC, :].reshape(60, NCH, NB, 24)
        xb = np.ascontiguousarray(xb.transpose(0, 1, 3, 2))
        percore.append({"xa": xa, "xb": xb})

    # conv1 stationaries for the footprint tiles
    w1a = np.zeros((100, 256), np.float32)
    w1b = np.zeros((60, 128), np.float32)
    for r in range(5):
        for dx in range(5):
            for j in range(8):
                for o in range(16):
                    w1a[r * 20 + 2 * j + dx, j * 16 + o] = conv1_w[o, 0, r, dx]
                    w1a[r * 20 + 2 * j + 1 + dx, 128 + j * 16 + o] = conv1_w[o, 0, r, dx]
            for j in range(8, 12):
                for o in range(16):
                    w1b[r * 12 + 2 * (j - 8) + dx, (j - 8) * 16 + o] = conv1_w[o, 0, r, dx]
                    w1b[r * 12 + 2 * (j - 8) + 1 + dx, 64 + (j - 8) * 16 + o] = conv1_w[o, 0, r, dx]

    # conv2 toeplitz: pooled row layout p = j*16 + c (pp0: j<8), (j-8)*16+c (pp1)
    # w2p0 [3, 128, 256]: r taps, cols [Mc0 | Mc1]
    w2p0 = np.zeros((3, 128, 256), np.float32)
    w2p1mc1 = np.zeros((3, 64, 128), np.float32)
    w2p1mc2 = np.zeros((4, 64, 128), np.float32)
    for r in range(3):
        for j in range(8):
            for c in range(16):
                p = j * 16 + c
                for mci, xobase in ((0, 0), (1, 4)):
                    for xo in range(xobase, xobase + 4):
                        dx = j - xo
                        if 0 <= dx < 3:
                            for o2 in range(32):
                                w2p0[r, p, 128 * mci + (xo - xobase) * 32 + o2] = \
                                    conv2_w[o2, c, r, dx]
        for j in range(8, 12):
            for c in range(16):
                p = (j - 8) * 16 + c
                for xo in range(4, 8):
                    dx = j - xo
                    if 0 <= dx < 3:
                        for o2 in range(32):
                            w2p1mc1[r, p, (xo - 4) * 32 + o2] = conv2_w[o2, c, r, dx]
    for rr in range(4):
        for b_ in range(2):
            r = rr - b_
            if not (0 <= r < 3):
                continue
            for j in range(8, 12):
                for c in range(16):
                    p = (j - 8) * 16 + c
                    for xo in range(8, 10):
                        dx = j - xo
                        if 0 <= dx < 3:
                            for o2 in range(32):
                                w2p1mc2[rr, p, 64 * b_ + (xo - 8) * 32 + o2] = \
                                    conv2_w[o2, c, r, dx]

    # h feature permutation: our flat index (tile*128+part) -> reference f = o2*100 + y*10 + xo
    perm = np.zeros(3200, np.int64)
    for P in range(5):
        y0, y1 = 2 * P, 2 * P + 1
        tiles = []
        for yy in (y0, y1):
            for xobase in (0, 4):
                tiles.append([(yy, xo, o2) for xo in range(xobase, xobase + 4)
                              for o2 in range(32)])
        t4 = [(y0, xo, o2) for xo in range(8, 10) for o2 in range(32)] + \
             [(y1, xo, o2) for xo in range(8, 10) for o2 in range(32)]
        order = [tiles[0], tiles[1], tiles[2], tiles[3], t4]
        for ti, tl in enumerate(order):
            for p, (yy, xo, o2) in enumerate(tl):
                perm[(5 * P + ti) * 128 + p] = o2 * 100 + yy * 10 + xo
    e1wpk = e1_w[:, perm, :].reshape(NE, 25, 128, DH)          # [E,kc,128,DH]
    e1wr = np.ascontiguousarray(e1wpk.transpose(0, 2, 1, 3)).reshape(NE, 128, 25 * DH)
    gwp = gate_w[perm, :].reshape(25, 128, NE).astype(np.float32)

    b1col = np.asarray(conv1_b, np.float32)[np.arange(128) % 16].reshape(128, 1)
    b2col = np.asarray(conv2_b, np.float32)[np.arange(128) % 32].reshape(128, 1)
    gbcol = np.asarray(gate_b, np.float32).reshape(NE, 1)
    e1bT = np.asarray(e1_b, np.float32).T.copy()      # [128, 5]
    e2bT = np.asarray(e2_b, np.float32).T.copy()      # [128, 5]
    smw = np.asarray(sm_w, np.float32)                # [128, 10]
    smb5 = np.tile(np.asarray(sm_b, np.float32), 5).reshape(1, 50)

    weights = dict(
        w1a=w1a.astype(bf), w1b=w1b.astype(bf),
        w2p0=np.ascontiguousarray(w2p0.transpose(1, 0, 2)).reshape(128, 768).astype(bf),
        w2p1mc1=np.ascontiguousarray(w2p1mc1.transpose(1, 0, 2)).reshape(64, 384).astype(bf),
        w2p1mc2=np.ascontiguousarray(w2p1mc2.transpose(1, 0, 2)).reshape(64, 512).astype(bf),
        e1wr=e1wr.astype(bf),
        gwp=np.ascontiguousarray(gwp.transpose(1, 0, 2)).reshape(128, 125).astype(bf),
        e2w=np.ascontiguousarray(e2_w.astype(np.float32).transpose(1, 0, 2)).reshape(128, 640).astype(bf),
        b1col=b1col, b2col=b2col,
        gbcol=gbcol, e1bT=e1bT, e2bT=e2bT, smw=smw.astype(bf), smb5=smb5.astype(bf))
    return percore, weights


def _build_nc(loop_reps=None):
    nc = bacc.Bacc("TRN2", target_bir_lowering=False, debug=False)
    d = {}
    def din(name, shape, dt):
        d[name] = nc.dram_tensor(name, list(shape), dt, kind="ExternalInput").ap()
    din("xa", (100, NCH, 24, NB), BF16)
    din("xb", (60, NCH, 24, NB), BF16)
    din("w1a", (100, 256), BF16)
    din("w1b", (60, 128), BF16)
    din("w2p0", (128, 768), BF16)
    din("w2p1mc1", (64, 384), BF16)
    din("w2p1mc2", (64, 512), BF16)
    din("e1wr", (NE, 128, 25 * DH), BF16)
    din("gwp", (128, 125), BF16)
    din("e2w", (128, 640), BF16)
    din("b1col", (128, 1), F32)
    din("b2col", (128, 1), F32)
    din("gbcol", (NE, 1), F32)
    din("e1bT", (128, NE), F32)
    din("e2bT", (128, NE), F32)
    din("smw", (128, 10), BF16)
    din("smb5", (1, 50), BF16)
    out_d = nc.dram_tensor("out", [BC, 10], F32, kind="ExternalOutput").ap()

    with tile.TileContext(nc) as tc:
        _emit(nc, tc, d, out_d, loop_reps=loop_reps)
    nc.compile()
    return nc


def _emit(nc, tc, d, out_d, loop_reps=None):
    ctx = contextlib.ExitStack()
    with ctx:
        wpool = ctx.enter_context(tc.tile_pool(name="wpool", bufs=1))
        xtp = ctx.enter_context(tc.tile_pool(name="xtp", bufs=1))
        tmp = ctx.enter_context(tc.tile_pool(name="tmp", bufs=2))
        shp = ctx.enter_context(tc.tile_pool(name="shp", bufs=2))
        pp0p = ctx.enter_context(tc.tile_pool(name="pp0p", bufs=12))
        pp1p = ctx.enter_context(tc.tile_pool(name="pp1p", bufs=12))
        hpool = ctx.enter_context(tc.tile_pool(name="hpool", bufs=25))
        h1p = ctx.enter_context(tc.tile_pool(name="h1p", bufs=2))
        h2p = ctx.enter_context(tc.tile_pool(name="h2p", bufs=5))
        gp = ctx.enter_context(tc.tile_pool(name="gp", bufs=2))
        smallp = ctx.enter_context(tc.tile_pool(name="smallp", bufs=8))
        c1ps = ctx.enter_context(tc.tile_pool(name="c1ps", bufs=4, space="PSUM"))
        c2ps = ctx.enter_context(tc.tile_pool(name="c2ps", bufs=2, space="PSUM"))
        mps = ctx.enter_context(tc.tile_pool(name="mps", bufs=2, space="PSUM"))

        # resident weights
        w1a = wpool.tile([100, 256], BF16); nc.sync.dma_start(w1a[:], d["w1a"][:])
        w1b = wpool.tile([60, 128], BF16); nc.sync.dma_start(w1b[:], d["w1b"][:])
        w2p0 = wpool.tile([128, 3 * 256], BF16)
        nc.sync.dma_start(w2p0[:], d["w2p0"][:])
        w2p1a = wpool.tile([64, 3 * 128], BF16)
        nc.sync.dma_start(w2p1a[:], d["w2p1mc1"][:])
        w2p1b = wpool.tile([64, 4 * 128], BF16)
        nc.sync.dma_start(w2p1b[:], d["w2p1mc2"][:])
        gw = wpool.tile([128, 25 * NE], BF16)
        nc.sync.dma_start(gw[:], d["gwp"][:])
        e2w = wpool.tile([128, NE * DH], BF16)
        nc.sync.dma_start(e2w[:], d["e2w"][:])
        e1wt = []
        for e in range(NE):
            t = wpool.tile([128, 25 * DH], BF16, tag=f"e1w{e}", name=f"e1w{e}")
            nc.sync.dma_start(t[:], d["e1wr"][e])
            e1wt.append(t)
        b1c = wpool.tile([128, 1], F32); nc.sync.dma_start(b1c[:], d["b1col"][:])
        b2c = wpool.tile([128, 1], F32); nc.sync.dma_start(b2c[:], d["b2col"][:])
        gbc = wpool.tile([NE, 1], F32); nc.sync.dma_start(gbc[:], d["gbcol"][:])
        e1bT = wpool.tile([128, NE], F32); nc.sync.dma_start(e1bT[:], d["e1bT"][:])
        e2bT = wpool.tile([128, NE], F32); nc.sync.dma_start(e2bT[:], d["e2bT"][:])
        smw = wpool.tile([128, 10], BF16); nc.sync.dma_start(smw[:], d["smw"][:])
        smb5 = wpool.tile([1, 50], BF16); nc.sync.dma_start(smb5[:], d["smb5"][:])
        ident = wpool.tile([128, 128], F32)
        make_identity(nc, ident[:])
        ones = wpool.tile([1, 128], BF16)
        nc.scalar.activation(ones[:], e2w[0:1, 0:128], AF.Copy, scale=0.0, bias=1.0)

        loop_cm = tc.For_i(0, loop_reps, 1) if loop_reps else contextlib.nullcontext()
        with loop_cm:
         for ch in range(NCH):
            b0 = ch * NB
            tA = xtp.tile([100, 24, NB], BF16, tag="tA")
            nc.sync.dma_start(tA[:], d["xa"][:, ch])
            tB = xtp.tile([60, 24, NB], BF16, tag="tB")
            nc.sync.dma_start(tB[:], d["xb"][:, ch])

            pp0, pp1, htiles = [], [], []

            def conv2_block(P):
                y0 = 2 * P
                for yy in (y0, y0 + 1):
                    for mci in range(2):
                        ps = c2ps.tile([128, NB], F32, tag="ps", name="c2t")
                        for r in range(3):
                            nc.tensor.matmul(
                                ps[:], w2p0[:, 256 * r + 128 * mci:256 * r + 128 * mci + 128],
                                pp0[yy + r][:], start=(r == 0),
                                stop=(mci == 0 and r == 2))
                        if mci == 1:
                            for r in range(3):
                                nc.tensor.matmul(ps[:], w2p1a[:, 128 * r:128 * r + 128],
                                                 pp1[yy + r][:], start=False,
                                                 stop=(r == 2))
                        h = hpool.tile([128, NB], BF16, tag="h", name="ht")
                        if mci == 1:
                            nc.vector.tensor_scalar(h[:], ps[:], b2c[:, 0:1], 0.0,
                                                    op0=ALU.add, op1=ALU.max)
                        else:
                            nc.scalar.activation(h[:], ps[:], AF.Relu, bias=b2c[:, 0:1])
                        htiles.append(h)
                ps = c2ps.tile([128, NB], F32, tag="ps", name="c2t")
                for rr in range(4):
                    nc.tensor.matmul(ps[:], w2p1b[:, 128 * rr:128 * rr + 128],
                                     pp1[y0 + rr][:], start=(rr == 0), stop=(rr == 3))
                h = hpool.tile([128, NB], BF16, tag="h", name="ht")
                nc.scalar.activation(h[:], ps[:], AF.Relu, bias=b2c[:, 0:1])
                htiles.append(h)

            # ---- conv1 (one matmul per tile) + pool, conv2 interleaved ----
            for Y in range(12):
                y0, y1 = 2 * Y, 2 * Y + 1
                tms = []
                for wsl, dat, tg in ((w1a[:, 0:128], tA, "tm0"),
                                     (w1a[:, 128:256], tA, "tm2"),
                                     (w1b[:, 0:128], tB, "tm1")):
                    pa = c1ps.tile([128, NB], F32, tag="ps", name="c1t")
                    nc.tensor.matmul(pa[:], wsl, dat[:, y0, :], start=True, stop=True)
                    pb = c1ps.tile([128, NB], F32, tag="ps", name="c1t")
                    nc.tensor.matmul(pb[:], wsl, dat[:, y1, :], start=True, stop=True)
                    ra = tmp.tile([128, NB], BF16, tag=tg + "r", name="rat")
                    nc.scalar.activation(ra[:], pa[:], AF.Copy)
                    tm = tmp.tile([128, NB], BF16, tag=tg, name="tmt")
                    nc.vector.tensor_tensor(tm[:], pb[:], ra[:], op=ALU.max)
                    tms.append(tm)
                tm0, tm2, tm1 = tms
                p0 = tmp.tile([128, NB], BF16, tag="p0", name="p0t")
                nc.vector.tensor_tensor(p0[:], tm0[:], tm2[:], op=ALU.max)
                sh = shp.tile([64, NB], BF16, tag="sh", name="sht")
                nc.sync.dma_start(sh[:], tm1[64:128, :])
                p1 = tmp.tile([64, NB], BF16, tag="p1", name="p1t")
                nc.vector.tensor_tensor(p1[:], tm1[0:64, :], sh[:], op=ALU.max)
                t0 = pp0p.tile([128, NB], BF16, tag="pp0", name="pp0t")
                nc.gpsimd.tensor_scalar(t0[:], p0[:], b1c[:, 0:1], 0.0,
                                        op0=ALU.add, op1=ALU.max)
                t1 = pp1p.tile([64, NB], BF16, tag="pp1", name="pp1t")
                nc.gpsimd.tensor_scalar(t1[:], p1[:], b1c[0:64, 0:1], 0.0,
                                        op0=ALU.add, op1=ALU.max)
                pp0.append(t0)
                pp1.append(t1)
                if Y >= 3 and Y % 2 == 1:
                    conv2_block((Y - 3) // 2)

            # ---- gate ----
            gps = mps.tile([NE, NB], F32, tag="m", name="gpst")
            for kc in range(25):
                nc.tensor.matmul(gps[:], gw[:, NE * kc:NE * kc + NE], htiles[kc][:],
                                 start=(kc == 0), stop=(kc == 24))
            gsb = gp.tile([NE, NB], F32, tag="gsb")
            nc.scalar.activation(gsb[:], gps[:], AF.Identity, bias=gbc[:, 0:1])

            # ---- experts (h2 of expert e-1 interleaved under e's h1) ----
            h1s = [None] * NE
            h2t = [None] * NE

            def h2_for(e):
                h2ps = mps.tile([128, NB], F32, tag="m", name="h2pst")
                nc.tensor.matmul(h2ps[:], e2w[:, DH * e:DH * e + DH], h1s[e][:],
                                 start=True, stop=True)
                h2 = h2p.tile([128, NB], BF16, tag="h2", name="h2t_")
                nc.scalar.activation(h2[:], h2ps[:], AF.Tanh, bias=e2bT[:, e:e + 1])
                h2t[e] = h2

            for e in range(NE):
                h1ps = mps.tile([128, NB], F32, tag="m", name="h1pst")
                for kc in range(25):
                    nc.tensor.matmul(h1ps[:], e1wt[e][:, DH * kc:DH * kc + DH],
                                     htiles[kc][:], start=(kc == 0), stop=(kc == 24))
                h1 = h1p.tile([128, NB], BF16, tag="h1", name="h1t_")
                nc.scalar.activation(h1[:], h1ps[:], AF.Tanh, bias=e1bT[:, e:e + 1])
                h1s[e] = h1
                if e >= 1:
                    h2_for(e - 1)
            h2_for(NE - 1)

            # ---- per-token-chunk: gating weights, head, softmax ----
            for t4 in range(NB // 128):
                tok = slice(128 * t4, 128 * t4 + 128)
                gtp = mps.tile([128, NE], F32, tag="m", name="gtpt")
                nc.tensor.transpose(gtp[:], gsb[:, tok], ident[0:NE, 0:NE])
                s = smallp.tile([128, NE], F32, tag="s")
                nc.scalar.activation(s[:], gtp[:], AF.Copy)
                mx = smallp.tile([128, 1], F32, tag="mx")
                nc.vector.reduce_max(mx[:], s[:], axis=AX.X)
                nmx = smallp.tile([128, 1], F32, tag="nmx")
                nc.vector.tensor_scalar_mul(nmx[:], mx[:], -1.0)
                ex = smallp.tile([128, NE], F32, tag="ex")
                nc.scalar.activation(ex[:], s[:], AF.Exp, bias=nmx[:, 0:1])
                gt = smallp.tile([128, NE * NE], F32, tag="gt")
                a_b = ex[:].unsqueeze(1).broadcast_to([128, NE, NE])
                b_b = ex[:].unsqueeze(2).broadcast_to([128, NE, NE])
                nc.vector.tensor_tensor(gt[:].rearrange("p (i j) -> p i j", i=NE),
                                        a_b, b_b, op=ALU.is_gt)
                rank = smallp.tile([128, NE], F32, tag="rank")
                nc.vector.reduce_sum(rank[:], gt[:].rearrange("p (i j) -> p i j", i=NE),
                                     axis=AX.X)
                m01 = smallp.tile([128, NE], F32, tag="m01")
                nc.vector.tensor_scalar(m01[:], rank[:], float(TOPK) - 0.5, None,
                                        op0=ALU.is_le)
                wun = smallp.tile([128, NE], F32, tag="wun")
                nc.vector.tensor_mul(wun[:], ex[:], m01[:])
                ssum = smallp.tile([128, 1], F32, tag="ssum")
                nc.vector.reduce_sum(ssum[:], wun[:], axis=AX.X)
                rinv = smallp.tile([128, 1], F32, tag="rinv")
                nc.vector.reciprocal(rinv[:], ssum[:])
                wfin = smallp.tile([128, NE], F32, tag="wfin")
                nc.vector.tensor_scalar_mul(wfin[:], wun[:], rinv[:, 0:1])

                lep = mps.tile([128, 50], F32, tag="m", name="lept")
                nc.tensor.matmul(lep[:, 0:50], ones[:], smb5[:], start=True, stop=False)
                for e in range(NE):
                    nc.tensor.matmul(lep[:, 10 * e:10 * e + 10], h2t[e][:, tok],
                                     smw[:], start=False, stop=(e == NE - 1))
                scl = smallp.tile([128, 50], F32, tag="scl")
                for e in range(NE):
                    nc.vector.tensor_scalar_mul(scl[:, 10 * e:10 * e + 10],
                                                lep[:, 10 * e:10 * e + 10],
                                                wfin[:, e:e + 1])
                logit = smallp.tile([128, 10], F32, tag="logit")
                nc.vector.reduce_sum(logit[:], scl[:].rearrange("p (e k) -> p k e", e=NE),
                                     axis=AX.X)
                lmx = smallp.tile([128, 1], F32, tag="lmx")
                nc.vector.reduce_max(lmx[:], logit[:], axis=AX.X)
                nlmx = smallp.tile([128, 1], F32, tag="nlmx")
                nc.vector.tensor_scalar_mul(nlmx[:], lmx[:], -1.0)
                lex = smallp.tile([128, 10], F32, tag="lex")
                nc.scalar.activation(lex[:], logit[:], AF.Exp, bias=nlmx[:, 0:1])
                lsum = smallp.tile([128, 1], F32, tag="lsum")
                nc.vector.reduce_sum(lsum[:], lex[:], axis=AX.X)
                lrinv = smallp.tile([128, 1], F32, tag="lrinv")
                nc.vector.reciprocal(lrinv[:], lsum[:])
                prob = smallp.tile([128, 10], F32, tag="prob")
                nc.vector.tensor_scalar_mul(prob[:], lex[:], lrinv[:, 0:1])
                nc.sync.dma_start(out_d[b0 + 128 * t4:b0 + 128 * t4 + 128, :], prob[:])


_NC_CACHE = None


def _get_nc():
    global _NC_CACHE
    if _NC_CACHE is None:
        _NC_CACHE = _build_nc()
    return _NC_CACHE


def kernel(**inputs):
    percore, w = _host_prep(**inputs)
    in_maps = [{**percore[c], **w} for c in range(NCORES)]
    nc = _get_nc()
    trace = bool(int(os.environ.get("KERNEL_TRACE", "0")))
    res = run_bass_kernel_spmd(nc, in_maps, list(range(NCORES)), trace=trace)
    kernel.last_results = res
    out = np.concatenate([res.results[c]["out"] for c in range(NCORES)], axis=0)
    return out.astype(np.float32)


# revision 9
# speedup vs baseline: 1.7558x; 1.7558x over previous
"""Trainium2 Bass kernel for nn_ClassifyModelMOE (conv feature extractor +
top-3-of-5 MoE + softmax head). Data-parallel over batch across 8 cores.

Self-contained: hardcodes all shapes; builds footprint-packed conv1 inputs and
Toeplitz-expanded conv weights on the host; runs one SPMD Bass/Tile program on
cores 0-7 via run_bass_kernel_spmd.

Structure per 512-token chunk:
  - 2 batched DMAs bring in footprint-packed x tiles (tA [100,24,512],
    tB [60,24,512]); conv1 is ONE matmul per output tile (K=95..100).
  - max-pool reads conv1 PSUM pairs directly (DVE), horizontal max on GpSimd,
    relu+bias fused into a DVE tensor_scalar on the pooled data.
  - conv2 blocks are interleaved into the conv1 row loop to keep PE fed.
  - expert-1 weights are SBUF-resident (loaded once).
"""
import os
import sys
import contextlib

sys.path.insert(0, "/opt/trn_rl_repo")

import numpy as np
import ml_dtypes

import concourse.bacc as bacc
import concourse.mybir as mybir
import concourse.tile as tile
from concourse.bass_utils import run_bass_kernel_spmd
from concourse.masks import make_identity

F32 = mybir.dt.float32
BF16 = mybir.dt.bfloat16
AF = mybir.ActivationFunctionType
ALU = mybir.AluOpType
AX = mybir.AxisListType

NCORES = 8
B = 8192
BC = B // NCORES          # tokens per core
NB = 512                  # batch chunk (columns per matmul)
NCH = BC // NB            # chunks per core
NE, TOPK = 5, 3
DH = 128

# conv1 output geometry: 16ch x 24x24; M-layout (per output row y):
#   Mc0: even x = 2j, j=0..7   -> m = j*16 + o        (128)
#   Mc2: odd x = 2j+1, j=0..7  -> m = j*16 + o        (128)
#   Mc1: x=16..23: [even j=8..11 | odd j=8..11]       (64+64)
# conv1 K-packing (footprint tiles):
#   tA part p = r*20 + c  <-> x[y+r, c],    c=0..19  (serves Mc0 and Mc2)
#   tB part p = r*12 + cc <-> x[y+r, 16+cc], cc=0..11 (serves Mc1)
# pooled row tiles: pp0 = j 0..7 (128 parts: j*16+c), pp1 = j 8..11 (64 parts)
# conv2 output (per row y): M = xout*32 + o2:
#   Mc0: xout 0..3 (128), Mc1: xout 4..7 (128),
#   Mc2pair: [y0: xout 8..9 | y1: xout 8..9] (64+64)


def _host_prep(x, conv1_w, conv1_b, conv2_w, conv2_b, gate_w, gate_b,
               e1_w, e1_b, e2_w, e2_b, sm_w, sm_b):
    bf = ml_dtypes.bfloat16
    x = np.asarray(x, np.float32)
    conv1_w = np.asarray(conv1_w, np.float32)
    conv2_w = np.asarray(conv2_w, np.float32)
    gate_w = np.asarray(gate_w, np.float32)
    e1_w = np.asarray(e1_w, np.float32)
    e2_w = np.asarray(e2_w, np.float32)

    xr = x.reshape(B, 28, 28)

    # footprint-packed x for conv1 (host-duplicated, per-core sliced below)
    xa_full = np.empty((100, B, 24), np.float32)
    for r in range(5):
        for c in range(20):
            xa_full[r * 20 + c] = xr[:, r:r + 24, c]
    xb_full = np.empty((60, B, 24), np.float32)
    for r in range(5):
        for cc in range(12):
            xb_full[r * 12 + cc] = xr[:, r:r + 24, 16 + cc]
    xa_full = xa_full.astype(bf)
    xb_full = xb_full.astype(bf)

    percore = []
    for cid in range(NCORES):
        c0 = cid * BC
        xa = xa_full[:, c0:c0 + BC, :].reshape(100, NCH, NB, 24)
        xa = np.ascontiguousarray(xa.transpose(0, 1, 3, 2))   # [100,NCH,24,NB]
        xb = xb_full[:, c0:c0 + BC, :].reshape(60, NCH, NB, 24)
        xb = np.ascontiguousarray(xb.transpose(0, 1, 3, 2))
        percore.append({"xa": xa, "xb": xb})

    # conv1 stationaries for the footprint tiles
    w1a = np.zeros((100, 256), np.float32)
    w1b = np.zeros((60, 128), np.float32)
    for r in range(5):
        for dx in range(5):
            for j in range(8):
                for o in range(16):
                    w1a[r * 20 + 2 * j + dx, j * 16 + o] = conv1_w[o, 0, r, dx]
                    w1a[r * 20 + 2 * j + 1 + dx, 128 + j * 16 + o] = conv1_w[o, 0, r, dx]
            for j in range(8, 12):
                for o in range(16):
                    w1b[r * 12 + 2 * (j - 8) + dx, (j - 8) * 16 + o] = conv1_w[o, 0, r, dx]
                    w1b[r * 12 + 2 * (j - 8) + 1 + dx, 64 + (j - 8) * 16 + o] = conv1_w[o, 0, r, dx]

    # conv2 toeplitz: pooled row layout p = j*16 + c (pp0: j<8), (j-8)*16+c (pp1)
    # w2p0 [3, 128, 256]: r taps, cols [Mc0 | Mc1]
    w2p0 = np.zeros((3, 128, 256), np.float32)
    w2p1mc1 = np.zeros((3, 64, 128), np.float32)
    w2p1mc2 = np.zeros((4, 64, 128), np.float32)
    for r in range(3):
        for j in range(8):
            for c in range(16):
                p = j * 16 + c
                for mci, xobase in ((0, 0), (1, 4)):
                    for xo in range(xobase, xobase + 4):
                        dx = j - xo
                        if 0 <= dx < 3:
                            for o2 in range(32):
                                w2p0[r, p, 128 * mci + (xo - xobase) * 32 + o2] = \
                                    conv2_w[o2, c, r, dx]
        for j in range(8, 12):
            for c in range(16):
                p = (j - 8) * 16 + c
                for xo in range(4, 8):
                    dx = j - xo
                    if 0 <= dx < 3:
                        for o2 in range(32):
                            w2p1mc1[r, p, (xo - 4) * 32 + o2] = conv2_w[o2, c, r, dx]
    for rr in range(4):
        for b_ in range(2):
            r = rr - b_
            if not (0 <= r < 3):
                continue
            for j in range(8, 12):
                for c in range(16):
                    p = (j - 8) * 16 + c
                    for xo in range(8, 10):
                        dx = j - xo
                        if 0 <= dx < 3:
                            for o2 in range(32):
                                w2p1mc2[rr, p, 64 * b_ + (xo - 8) * 32 + o2] = \
                                    conv2_w[o2, c, r, dx]

    # h feature permutation: our flat index (tile*128+part) -> reference f = o2*100 + y*10 + xo
    perm = np.zeros(3200, np.int64)
    for P in range(5):
        y0, y1 = 2 * P, 2 * P + 1
        tiles = []
        for yy in (y0, y1):
            for xobase in (0, 4):
                tiles.append([(yy, xo, o2) for xo in range(xobase, xobase + 4)
                              for o2 in range(32)])
        t4 = [(y0, xo, o2) for xo in range(8, 10) for o2 in range(32)] + \
             [(y1, xo, o2) for xo in range(8, 10) for o2 in range(32)]
        order = [tiles[0], tiles[1], tiles[2], tiles[3], t4]
        for ti, tl in enumerate(order):
            for p, (yy, xo, o2) in enumerate(tl):
                perm[(5 * P + ti) * 128 + p] = o2 * 100 + yy * 10 + xo
    e1wpk = e1_w[:, perm, :].reshape(NE, 25, 128, DH)          # [E,kc,128,DH]
    e1wr = np.ascontiguousarray(e1wpk.transpose(0, 2, 1, 3)).reshape(NE, 128, 25 * DH)
    gwp = gate_w[perm, :].reshape(25, 128, NE).astype(np.float32)

    b1col = np.asarray(conv1_b, np.float32)[np.arange(128) % 16].reshape(128, 1)
    b2col = np.asarray(conv2_b, np.float32)[np.arange(128) % 32].reshape(128, 1)
    gbcol = np.asarray(gate_b, np.float32).reshape(NE, 1)
    e1bT = np.asarray(e1_b, np.float32).T.copy()      # [128, 5]
    e2bT = np.asarray(e2_b, np.float32).T.copy()      # [128, 5]
    smw = np.asarray(sm_w, np.float32)                # [128, 10]
    smb5 = np.tile(np.asarray(sm_b, np.float32), 5).reshape(1, 50)

    weights = dict(
        w1a=w1a.astype(bf), w1b=w1b.astype(bf),
        w2p0=np.ascontiguousarray(w2p0.transpose(1, 0, 2)).reshape(128, 768).astype(bf),
        w2p1mc1=np.ascontiguousarray(w2p1mc1.transpose(1, 0, 2)).reshape(64, 384).astype(bf),
        w2p1mc2=np.ascontiguousarray(w2p1mc2.transpose(1, 0, 2)).reshape(64, 512).astype(bf),
        e1wr=e1wr.astype(bf),
        gwp=np.ascontiguousarray(gwp.transpose(1, 0, 2)).reshape(128, 125).astype(bf),
        e2w=np.ascontiguousarray(e2_w.astype(np.float32).transpose(1, 0, 2)).reshape(128, 640).astype(bf),
        b1col=b1col, b2col=b2col,
        gbcol=gbcol, e1bT=e1bT, e2bT=e2bT, smw=smw.astype(bf), smb5=smb5.astype(bf))
    return percore, weights


def _build_nc(loop_reps=None):
    nc = bacc.Bacc("TRN2", target_bir_lowering=False, debug=False)
    d = {}
    def din(name, shape, dt):
        d[name] = nc.dram_tensor(name, list(shape), dt, kind="ExternalInput").ap()
    din("xa", (100, NCH, 24, NB), BF16)
    din("xb", (60, NCH, 24, NB), BF16)
    din("w1a", (100, 256), BF16)
    din("w1b", (60, 128), BF16)
    din("w2p0", (128, 768), BF16)
    din("w2p1mc1", (64, 384), BF16)
    din("w2p1mc2", (64, 512), BF16)
    din("e1wr", (NE, 128, 25 * DH), BF16)
    din("gwp", (128, 125), BF16)
    din("e2w", (128, 640), BF16)
    din("b1col", (128, 1), F32)
    din("b2col", (128, 1), F32)
    din("gbcol", (NE, 1), F32)
    din("e1bT", (128, NE), F32)
    din("e2bT", (128, NE), F32)
    din("smw", (128, 10), BF16)
    din("smb5", (1, 50), BF16)
    out_d = nc.dram_tensor("out", [BC, 10], F32, kind="ExternalOutput").ap()

    with tile.TileContext(nc) as tc:
        _emit(nc, tc, d, out_d, loop_reps=loop_reps)
    nc.compile()
    return nc


def _emit(nc, tc, d, out_d, loop_reps=None):
    ctx = contextlib.ExitStack()
    with ctx:
        wpool = ctx.enter_context(tc.tile_pool(name="wpool", bufs=1))
        xtp = ctx.enter_context(tc.tile_pool(name="xtp", bufs=1))
        tmp = ctx.enter_context(tc.tile_pool(name="tmp", bufs=2))
        shp = ctx.enter_context(tc.tile_pool(name="shp", bufs=2))
        pp0p = ctx.enter_context(tc.tile_pool(name="pp0p", bufs=12))
        pp1p = ctx.enter_context(tc.tile_pool(name="pp1p", bufs=12))
        hpool = ctx.enter_context(tc.tile_pool(name="hpool", bufs=25))
        h1p = ctx.enter_context(tc.tile_pool(name="h1p", bufs=2))
        h2p = ctx.enter_context(tc.tile_pool(name="h2p", bufs=5))
        gp = ctx.enter_context(tc.tile_pool(name="gp", bufs=2))
        smallp = ctx.enter_context(tc.tile_pool(name="smallp", bufs=8))
        c1ps = ctx.enter_context(tc.tile_pool(name="c1ps", bufs=4, space="PSUM"))
        c2ps = ctx.enter_context(tc.tile_pool(name="c2ps", bufs=2, space="PSUM"))
        mps = ctx.enter_context(tc.tile_pool(name="mps", bufs=2, space="PSUM"))

        # resident weights
        w1a = wpool.tile([100, 256], BF16); nc.sync.dma_start(w1a[:], d["w1a"][:])
        w1b = wpool.tile([60, 128], BF16); nc.sync.dma_start(w1b[:], d["w1b"][:])
        w2p0 = wpool.tile([128, 3 * 256], BF16)
        nc.sync.dma_start(w2p0[:], d["w2p0"][:])
        w2p1a = wpool.tile([64, 3 * 128], BF16)
        nc.sync.dma_start(w2p1a[:], d["w2p1mc1"][:])
        w2p1b = wpool.tile([64, 4 * 128], BF16)
        nc.sync.dma_start(w2p1b[:], d["w2p1mc2"][:])
        gw = wpool.tile([128, 25 * NE], BF16)
        nc.sync.dma_start(gw[:], d["gwp"][:])
        e2w = wpool.tile([128, NE * DH], BF16)
        nc.sync.dma_start(e2w[:], d["e2w"][:])
        e1wt = []
        for e in range(NE):
            t = wpool.tile([128, 25 * DH], BF16, tag=f"e1w{e}", name=f"e1w{e}")
            nc.sync.dma_start(t[:], d["e1wr"][e])
            e1wt.append(t)
        b1c = wpool.tile([128, 1], F32); nc.sync.dma_start(b1c[:], d["b1col"][:])
        b2c = wpool.tile([128, 1], F32); nc.sync.dma_start(b2c[:], d["b2col"][:])
        gbc = wpool.tile([NE, 1], F32); nc.sync.dma_start(gbc[:], d["gbcol"][:])
        e1bT = wpool.tile([128, NE], F32); nc.sync.dma_start(e1bT[:], d["e1bT"][:])
        e2bT = wpool.tile([128, NE], F32); nc.sync.dma_start(e2bT[:], d["e2bT"][:])
        smw = wpool.tile([128, 10], BF16); nc.sync.dma_start(smw[:], d["smw"][:])
        smb5 = wpool.tile([1, 50], BF16); nc.sync.dma_start(smb5[:], d["smb5"][:])
        ident = wpool.tile([128, 128], F32)
        make_identity(nc, ident[:])
        ones = wpool.tile([1, 128], BF16)
        nc.scalar.activation(ones[:], e2w[0:1, 0:128], AF.Copy, scale=0.0, bias=1.0)

        loop_cm = tc.For_i(0, loop_reps, 1) if loop_reps else contextlib.nullcontext()
        with loop_cm:
         for ch in range(NCH):
            b0 = ch * NB
            tA = xtp.tile([100, 24, NB], BF16, tag="tA")
            nc.sync.dma_start(tA[:], d["xa"][:, ch])
            tB = xtp.tile([60, 24, NB], BF16, tag="tB")
            nc.sync.dma_start(tB[:], d["xb"][:, ch])

            pp0, pp1, htiles = [], [], []

            def conv2_block(P):
                y0 = 2 * P
                for yy in (y0, y0 + 1):
                    for mci in range(2):
                        ps = c2ps.tile([128, NB], F32, tag="ps", name="c2t")
                        for r in range(3):
                            nc.tensor.matmul(
                                ps[:], w2p0[:, 256 * r + 128 * mci:256 * r + 128 * mci + 128],
                                pp0[yy + r][:], start=(r == 0),
                                stop=(mci == 0 and r == 2))
                        if mci == 1:
                            for r in range(3):
                                nc.tensor.matmul(ps[:], w2p1a[:, 128 * r:128 * r + 128],
                                                 pp1[yy + r][:], start=False,
                                                 stop=(r == 2))
                        h = hpool.tile([128, NB], BF16, tag="h", name="ht")
                        if mci == 1:
                            nc.vector.tensor_scalar(h[:], ps[:], b2c[:, 0:1], 0.0,
                                                    op0=ALU.add, op1=ALU.max)
                        else:
                            nc.scalar.activation(h[:], ps[:], AF.Relu, bias=b2c[:, 0:1])
                        htiles.append(h)
                ps = c2ps.tile([128, NB], F32, tag="ps", name="c2t")
                for rr in range(4):
                    nc.tensor.matmul(ps[:], w2p1b[:, 128 * rr:128 * rr + 128],
                                     pp1[y0 + rr][:], start=(rr == 0), stop=(rr == 3))
                h = hpool.tile([128, NB], BF16, tag="h", name="ht")
                nc.scalar.activation(h[:], ps[:], AF.Relu, bias=b2c[:, 0:1])
                htiles.append(h)

            # ---- conv1 (one matmul per tile) + pool, conv2 interleaved ----
            for Y in range(12):
                y0, y1 = 2 * Y, 2 * Y + 1
                tms = []
                for wsl, dat, tg in ((w1a[:, 0:128], tA, "tm0"),
                                     (w1a[:, 128:256], tA, "tm2"),
                                     (w1b[:, 0:128], tB, "tm1")):
                    pa = c1ps.tile([128, NB], F32, tag="ps", name="c1t")
                    nc.tensor.matmul(pa[:], wsl, dat[:, y0, :], start=True, stop=True)
                    pb = c1ps.tile([128, NB], F32, tag="ps", name="c1t")
                    nc.tensor.matmul(pb[:], wsl, dat[:, y1, :], start=True, stop=True)
                    ra = tmp.tile([128, NB], BF16, tag=tg + "r", name="rat")
                    nc.scalar.activation(ra[:], pa[:], AF.Copy)
                    tm = tmp.tile([128, NB], BF16, tag=tg, name="tmt")
                    nc.vector.tensor_tensor(tm[:], pb[:], ra[:], op=ALU.max)
                    tms.append(tm)
                tm0, tm2, tm1 = tms
                p0 = tmp.tile([128, NB], BF16, tag="p0", name="p0t")
                nc.vector.tensor_tensor(p0[:], tm0[:], tm2[:], op=ALU.max)
                sh = shp.tile([64, NB], BF16, tag="sh", name="sht")
                nc.sync.dma_start(sh[:], tm1[64:128, :])
                p1 = tmp.tile([64, NB], BF16, tag="p1", name="p1t")
                nc.vector.tensor_tensor(p1[:], tm1[0:64, :], sh[:], op=ALU.max)
                t0 = pp0p.tile([128, NB], BF16, tag="pp0", name="pp0t")
                nc.vector.tensor_scalar(t0[:], p0[:], b1c[:, 0:1], 0.0,
                                        op0=ALU.add, op1=ALU.max)
                t1 = pp1p.tile([64, NB], BF16, tag="pp1", name="pp1t")
                nc.vector.tensor_scalar(t1[:], p1[:], b1c[0:64, 0:1], 0.0,
                                        op0=ALU.add, op1=ALU.max)
                pp0.append(t0)
                pp1.append(t1)
                if Y >= 3 and Y % 2 == 1:
                    conv2_block((Y - 3) // 2)

            # ---- gate ----
            gps = mps.tile([NE, NB], F32, tag="m", name="gpst")
            for kc in range(25):
                nc.tensor.matmul(gps[:], gw[:, NE * kc:NE * kc + NE], htiles[kc][:],
                                 start=(kc == 0), stop=(kc == 24))
            gsb = gp.tile([NE, NB], F32, tag="gsb")
            nc.scalar.activation(gsb[:], gps[:], AF.Identity, bias=gbc[:, 0:1])

            # ---- experts (h2 of expert e-1 interleaved under e's h1) ----
            h1s = [None] * NE
            h2t = [None] * NE

            def h2_for(e):
                h2ps = mps.tile([128, NB], F32, tag="m", name="h2pst")
                nc.tensor.matmul(h2ps[:], e2w[:, DH * e:DH * e + DH], h1s[e][:],
                                 start=True, stop=True)
                h2 = h2p.tile([128, NB], BF16, tag="h2", name="h2t_")
                nc.scalar.activation(h2[:], h2ps[:], AF.Tanh, bias=e2bT[:, e:e + 1])
                h2t[e] = h2

            for e in range(NE):
                h1ps = mps.tile([128, NB], F32, tag="m", name="h1pst")
                for kc in range(25):
                    nc.tensor.matmul(h1ps[:], e1wt[e][:, DH * kc:DH * kc + DH],
                                     htiles[kc][:], start=(kc == 0), stop=(kc == 24))
                h1 = h1p.tile([128, NB], BF16, tag="h1", name="h1t_")
                nc.scalar.activation(h1[:], h1ps[:], AF.Tanh, bias=e1bT[:, e:e + 1])
                h1s[e] = h1
                if e >= 1:
                    h2_for(e - 1)
            h2_for(NE - 1)

            # ---- per-token-chunk: gating weights, head, softmax ----
            for t4 in range(NB // 128):
                tok = slice(128 * t4, 128 * t4 + 128)
                gtp = mps.tile([128, NE], F32, tag="m", name="gtpt")
                nc.tensor.transpose(gtp[:], gsb[:, tok], ident[0:NE, 0:NE])
                s = smallp.tile([128, NE], F32, tag="s")
                nc.scalar.activation(s[:], gtp[:], AF.Copy)
                mx = smallp.tile([128, 1], F32, tag="mx")
                nc.vector.reduce_max(mx[:], s[:], axis=AX.X)
                nmx = smallp.tile([128, 1], F32, tag="nmx")
                nc.vector.tensor_scalar_mul(nmx[:], mx[:], -1.0)
                ex = smallp.tile([128, NE], F32, tag="ex")
                nc.scalar.activation(ex[:], s[:], AF.Exp, bias=nmx[:, 0:1])
                gt = smallp.tile([128, NE * NE], F32, tag="gt")
                a_b = ex[:].unsqueeze(1).broadcast_to([128, NE, NE])
                b_b = ex[:].unsqueeze(2).broadcast_to([128, NE, NE])
                nc.vector.tensor_tensor(gt[:].rearrange("p (i j) -> p i j", i=NE),
                                        a_b, b_b, op=ALU.is_gt)
                rank = smallp.tile([128, NE], F32, tag="rank")
                nc.vector.reduce_sum(rank[:], gt[:].rearrange("p (i j) -> p i j", i=NE),
                                     axis=AX.X)
                m01 = smallp.tile([128, NE], F32, tag="m01")
                nc.vector.tensor_scalar(m01[:], rank[:], float(TOPK) - 0.5, None,
                                        op0=ALU.is_le)
                wun = smallp.tile([128, NE], F32, tag="wun")
                nc.vector.tensor_mul(wun[:], ex[:], m01[:])
                ssum = smallp.tile([128, 1], F32, tag="ssum")
                nc.vector.reduce_sum(ssum[:], wun[:], axis=AX.X)
                rinv = smallp.tile([128, 1], F32, tag="rinv")
                nc.vector.reciprocal(rinv[:], ssum[:])
                wfin = smallp.tile([128, NE], F32, tag="wfin")
                nc.vector.tensor_scalar_mul(wfin[:], wun[:], rinv[:, 0:1])

                lep = mps.tile([128, 50], F32, tag="m", name="lept")
                nc.tensor.matmul(lep[:, 0:50], ones[:], smb5[:], start=True, stop=False)
                for e in range(NE):
                    nc.tensor.matmul(lep[:, 10 * e:10 * e + 10], h2t[e][:, tok],
                                     smw[:], start=False, stop=(e == NE - 1))
                scl = smallp.tile([128, 50], F32, tag="scl")
                for e in range(NE):
                    nc.vector.tensor_scalar_mul(scl[:, 10 * e:10 * e + 10],
                                                lep[:, 10 * e:10 * e + 10],
                                                wfin[:, e:e + 1])
                logit = smallp.tile([128, 10], F32, tag="logit")
                nc.vector.reduce_sum(logit[:], scl[:].rearrange("p (e k) -> p k e", e=NE),
                                     axis=AX.X)
                lmx = smallp.tile([128, 1], F32, tag="lmx")
                nc.vector.reduce_max(lmx[:], logit[:], axis=AX.X)
                nlmx = smallp.tile([128, 1], F32, tag="nlmx")
                nc.vector.tensor_scalar_mul(nlmx[:], lmx[:], -1.0)
                lex = smallp.tile([128, 10], F32, tag="lex")
                nc.scalar.activation(lex[:], logit[:], AF.Exp, bias=nlmx[:, 0:1])
                lsum = smallp.tile([128, 1], F32, tag="lsum")
                nc.vector.reduce_sum(lsum[:], lex[:], axis=AX.X)
                lrinv = smallp.tile([128, 1], F32, tag="lrinv")
                nc.vector.reciprocal(lrinv[:], lsum[:])
                prob = smallp.tile([128, 10], F32, tag="prob")
                nc.vector.tensor_scalar_mul(prob[:], lex[:], lrinv[:, 0:1])
                nc.sync.dma_start(out_d[b0 + 128 * t4:b0 + 128 * t4 + 128, :], prob[:])


_NC_CACHE = None


def _get_nc():
    global _NC_CACHE
    if _NC_CACHE is None:
        _NC_CACHE = _build_nc()
    return _NC_CACHE


def kernel(**inputs):
    percore, w = _host_prep(**inputs)
    in_maps = [{**percore[c], **w} for c in range(NCORES)]
    nc = _get_nc()
    trace = bool(int(os.environ.get("KERNEL_TRACE", "0")))
    res = run_bass_kernel_spmd(nc, in_maps, list(range(NCORES)), trace=trace)
    kernel.last_results = res
    out = np.concatenate([res.results[c]["out"] for c in range(NCORES)], axis=0)
    return out.astype(np.float32)


# revision 11
# speedup vs baseline: 2.0370x; 1.1601x over previous
"""Trainium2 Bass kernel for nn_ClassifyModelMOE (conv feature extractor +
top-3-of-5 MoE + softmax head). Data-parallel over batch across 8 cores.

Self-contained: hardcodes all shapes; builds footprint-packed conv1 inputs and
Toeplitz-expanded conv weights on the host; runs one SPMD Bass/Tile program on
cores 0-7 via run_bass_kernel_spmd.

Structure per 512-token chunk:
  - 2 batched DMAs bring in footprint-packed x tiles (tA [100,24,512],
    tB [60,24,512]); conv1 is ONE matmul per output tile (K=95..100).
  - max-pool reads conv1 PSUM pairs directly (DVE), horizontal max on GpSimd,
    relu+bias fused into a DVE tensor_scalar on the pooled data.
  - conv2 blocks are interleaved into the conv1 row loop to keep PE fed.
  - expert-1 weights are SBUF-resident (loaded once).
"""
import os
import sys
import contextlib

sys.path.insert(0, "/opt/trn_rl_repo")

import numpy as np
import ml_dtypes

import concourse.bacc as bacc
import concourse.mybir as mybir
import concourse.tile as tile
from concourse.bass_utils import run_bass_kernel_spmd
from concourse.masks import make_identity

F32 = mybir.dt.float32
BF16 = mybir.dt.bfloat16
AF = mybir.ActivationFunctionType
ALU = mybir.AluOpType
AX = mybir.AxisListType

NCORES = 8
B = 8192
BC = B // NCORES          # tokens per core
NB = 512                  # batch chunk (columns per matmul)
NCH = BC // NB            # chunks per core
NE, TOPK = 5, 3
DH = 128

# conv1 output geometry: 16ch x 24x24; M-layout (per output row y):
#   Mc0: even x = 2j, j=0..7   -> m = j*16 + o        (128)
#   Mc2: odd x = 2j+1, j=0..7  -> m = j*16 + o        (128)
#   Mc1: x=16..23: [even j=8..11 | odd j=8..11]       (64+64)
# conv1 K-packing (footprint tiles):
#   tA part p = r*20 + c  <-> x[y+r, c],    c=0..19  (serves Mc0 and Mc2)
#   tB part p = r*12 + cc <-> x[y+r, 16+cc], cc=0..11 (serves Mc1)
# pooled row tiles: pp0 = j 0..7 (128 parts: j*16+c), pp1 = j 8..11 (64 parts)
# conv2 output (per row y): M = xout*32 + o2:
#   Mc0: xout 0..3 (128), Mc1: xout 4..7 (128),
#   Mc2pair: [y0: xout 8..9 | y1: xout 8..9] (64+64)


def _host_prep(x, conv1_w, conv1_b, conv2_w, conv2_b, gate_w, gate_b,
               e1_w, e1_b, e2_w, e2_b, sm_w, sm_b):
    bf = ml_dtypes.bfloat16
    x = np.asarray(x, np.float32)
    conv1_w = np.asarray(conv1_w, np.float32)
    conv2_w = np.asarray(conv2_w, np.float32)
    gate_w = np.asarray(gate_w, np.float32)
    e1_w = np.asarray(e1_w, np.float32)
    e2_w = np.asarray(e2_w, np.float32)

    xr = x.reshape(B, 28, 28)

    # footprint-packed x for conv1 (host-duplicated, per-core sliced below)
    xa_full = np.empty((100, B, 24), np.float32)
    for r in range(5):
        for c in range(20):
            xa_full[r * 20 + c] = xr[:, r:r + 24, c]
    xb_full = np.empty((60, B, 24), np.float32)
    for r in range(5):
        for cc in range(12):
            xb_full[r * 12 + cc] = xr[:, r:r + 24, 16 + cc]
    xa_full = xa_full.astype(bf)
    xb_full = xb_full.astype(bf)

    percore = []
    for cid in range(NCORES):
        c0 = cid * BC
        xa = xa_full[:, c0:c0 + BC, :].reshape(100, NCH, NB, 24)
        xa = np.ascontiguousarray(xa.transpose(0, 1, 3, 2))   # [100,NCH,24,NB]
        xb = xb_full[:, c0:c0 + BC, :].reshape(60, NCH, NB, 24)
        xb = np.ascontiguousarray(xb.transpose(0, 1, 3, 2))
        percore.append({"xa": xa, "xb": xb})

    # conv1 stationaries for the footprint tiles
    w1a = np.zeros((100, 256), np.float32)
    w1b = np.zeros((60, 128), np.float32)
    for r in range(5):
        for dx in range(5):
            for j in range(8):
                for o in range(16):
                    w1a[r * 20 + 2 * j + dx, j * 16 + o] = conv1_w[o, 0, r, dx]
                    w1a[r * 20 + 2 * j + 1 + dx, 128 + j * 16 + o] = conv1_w[o, 0, r, dx]
            for j in range(8, 12):
                for o in range(16):
                    w1b[r * 12 + 2 * (j - 8) + dx, (j - 8) * 16 + o] = conv1_w[o, 0, r, dx]
                    w1b[r * 12 + 2 * (j - 8) + 1 + dx, 64 + (j - 8) * 16 + o] = conv1_w[o, 0, r, dx]

    # conv2 toeplitz: pooled row layout p = j*16 + c (pp0: j<8), (j-8)*16+c (pp1)
    # w2p0 [3, 128, 256]: r taps, cols [Mc0 | Mc1]
    w2p0 = np.zeros((3, 128, 256), np.float32)
    w2p1mc1 = np.zeros((3, 64, 128), np.float32)
    w2p1mc2 = np.zeros((4, 64, 128), np.float32)
    for r in range(3):
        for j in range(8):
            for c in range(16):
                p = j * 16 + c
                for mci, xobase in ((0, 0), (1, 4)):
                    for xo in range(xobase, xobase + 4):
                        dx = j - xo
                        if 0 <= dx < 3:
                            for o2 in range(32):
                                w2p0[r, p, 128 * mci + (xo - xobase) * 32 + o2] = \
                                    conv2_w[o2, c, r, dx]
        for j in range(8, 12):
            for c in range(16):
                p = (j - 8) * 16 + c
                for xo in range(4, 8):
                    dx = j - xo
                    if 0 <= dx < 3:
                        for o2 in range(32):
                            w2p1mc1[r, p, (xo - 4) * 32 + o2] = conv2_w[o2, c, r, dx]
    for rr in range(4):
        for b_ in range(2):
            r = rr - b_
            if not (0 <= r < 3):
                continue
            for j in range(8, 12):
                for c in range(16):
                    p = (j - 8) * 16 + c
                    for xo in range(8, 10):
                        dx = j - xo
                        if 0 <= dx < 3:
                            for o2 in range(32):
                                w2p1mc2[rr, p, 64 * b_ + (xo - 8) * 32 + o2] = \
                                    conv2_w[o2, c, r, dx]

    # h feature permutation: our flat index (tile*128+part) -> reference f = o2*100 + y*10 + xo
    perm = np.zeros(3200, np.int64)
    for P in range(5):
        y0, y1 = 2 * P, 2 * P + 1
        tiles = []
        for yy in (y0, y1):
            for xobase in (0, 4):
                tiles.append([(yy, xo, o2) for xo in range(xobase, xobase + 4)
                              for o2 in range(32)])
        t4 = [(y0, xo, o2) for xo in range(8, 10) for o2 in range(32)] + \
             [(y1, xo, o2) for xo in range(8, 10) for o2 in range(32)]
        order = [tiles[0], tiles[1], tiles[2], tiles[3], t4]
        for ti, tl in enumerate(order):
            for p, (yy, xo, o2) in enumerate(tl):
                perm[(5 * P + ti) * 128 + p] = o2 * 100 + yy * 10 + xo
    e1wpk = e1_w[:, perm, :].reshape(NE, 25, 128, DH)          # [E,kc,128,DH]
    e1wr = np.ascontiguousarray(e1wpk.transpose(0, 2, 1, 3)).reshape(NE, 128, 25 * DH)
    gwp = gate_w[perm, :].reshape(25, 128, NE).astype(np.float32)

    b1col = np.asarray(conv1_b, np.float32)[np.arange(128) % 16].reshape(128, 1)
    b2col = np.asarray(conv2_b, np.float32)[np.arange(128) % 32].reshape(128, 1)
    gbcol = np.asarray(gate_b, np.float32).reshape(NE, 1)
    e1bT = np.asarray(e1_b, np.float32).T.copy()      # [128, 5]
    e2bT = np.asarray(e2_b, np.float32).T.copy()      # [128, 5]
    smw = np.asarray(sm_w, np.float32)                # [128, 10]
    smb5 = np.tile(np.asarray(sm_b, np.float32), 5).reshape(1, 50)

    weights = dict(
        w1a=w1a.astype(bf), w1b=w1b.astype(bf),
        w2p0=np.ascontiguousarray(w2p0.transpose(1, 0, 2)).reshape(128, 768).astype(bf),
        w2p1mc1=np.ascontiguousarray(w2p1mc1.transpose(1, 0, 2)).reshape(64, 384).astype(bf),
        w2p1mc2=np.ascontiguousarray(w2p1mc2.transpose(1, 0, 2)).reshape(64, 512).astype(bf),
        e1wr=e1wr.astype(bf),
        gwp=np.ascontiguousarray(gwp.transpose(1, 0, 2)).reshape(128, 125).astype(bf),
        e2w=np.ascontiguousarray(e2_w.astype(np.float32).transpose(1, 0, 2)).reshape(128, 640).astype(bf),
        b1col=b1col, b2col=b2col,
        gbcol=gbcol, e1bT=e1bT, e2bT=e2bT, smw=smw.astype(bf), smb5=smb5.astype(bf))
    return percore, weights


def _build_nc(loop_reps=None):
    nc = bacc.Bacc("TRN2", target_bir_lowering=False, debug=False)
    d = {}
    def din(name, shape, dt):
        d[name] = nc.dram_tensor(name, list(shape), dt, kind="ExternalInput").ap()
    din("xa", (100, NCH, 24, NB), BF16)
    din("xb", (60, NCH, 24, NB), BF16)
    din("w1a", (100, 256), BF16)
    din("w1b", (60, 128), BF16)
    din("w2p0", (128, 768), BF16)
    din("w2p1mc1", (64, 384), BF16)
    din("w2p1mc2", (64, 512), BF16)
    din("e1wr", (NE, 128, 25 * DH), BF16)
    din("gwp", (128, 125), BF16)
    din("e2w", (128, 640), BF16)
    din("b1col", (128, 1), F32)
    din("b2col", (128, 1), F32)
    din("gbcol", (NE, 1), F32)
    din("e1bT", (128, NE), F32)
    din("e2bT", (128, NE), F32)
    din("smw", (128, 10), BF16)
    din("smb5", (1, 50), BF16)
    out_d = nc.dram_tensor("out", [BC, 10], F32, kind="ExternalOutput").ap()

    with tile.TileContext(nc) as tc:
        _emit(nc, tc, d, out_d, loop_reps=loop_reps)
    nc.compile()
    return nc


def _emit(nc, tc, d, out_d, loop_reps=None):
    ctx = contextlib.ExitStack()
    with ctx:
        wpool = ctx.enter_context(tc.tile_pool(name="wpool", bufs=1))
        xtp = ctx.enter_context(tc.tile_pool(name="xtp", bufs=1))
        tmp = ctx.enter_context(tc.tile_pool(name="tmp", bufs=2))
        shp = ctx.enter_context(tc.tile_pool(name="shp", bufs=2))
        pp0p = ctx.enter_context(tc.tile_pool(name="pp0p", bufs=12))
        pp1p = ctx.enter_context(tc.tile_pool(name="pp1p", bufs=12))
        hpool = ctx.enter_context(tc.tile_pool(name="hpool", bufs=25))
        h1p = ctx.enter_context(tc.tile_pool(name="h1p", bufs=2))
        h2p = ctx.enter_context(tc.tile_pool(name="h2p", bufs=5))
        gp = ctx.enter_context(tc.tile_pool(name="gp", bufs=2))
        smallp = ctx.enter_context(tc.tile_pool(name="smallp", bufs=8))
        c1ps = ctx.enter_context(tc.tile_pool(name="c1ps", bufs=4, space="PSUM"))
        c2ps = ctx.enter_context(tc.tile_pool(name="c2ps", bufs=2, space="PSUM"))
        mps = ctx.enter_context(tc.tile_pool(name="mps", bufs=2, space="PSUM"))

        # resident weights
        w1a = wpool.tile([100, 256], BF16); nc.sync.dma_start(w1a[:], d["w1a"][:])
        w1b = wpool.tile([60, 128], BF16); nc.sync.dma_start(w1b[:], d["w1b"][:])
        w2p0 = wpool.tile([128, 3 * 256], BF16)
        nc.sync.dma_start(w2p0[:], d["w2p0"][:])
        w2p1a = wpool.tile([64, 3 * 128], BF16)
        nc.sync.dma_start(w2p1a[:], d["w2p1mc1"][:])
        w2p1b = wpool.tile([64, 4 * 128], BF16)
        nc.sync.dma_start(w2p1b[:], d["w2p1mc2"][:])
        gw = wpool.tile([128, 25 * NE], BF16)
        nc.sync.dma_start(gw[:], d["gwp"][:])
        e2w = wpool.tile([128, NE * DH], BF16)
        nc.sync.dma_start(e2w[:], d["e2w"][:])
        e1wt = []
        for e in range(NE):
            t = wpool.tile([128, 25 * DH], BF16, tag=f"e1w{e}", name=f"e1w{e}")
            nc.sync.dma_start(t[:], d["e1wr"][e])
            e1wt.append(t)
        b1c = wpool.tile([128, 1], F32); nc.sync.dma_start(b1c[:], d["b1col"][:])
        b2c = wpool.tile([128, 1], F32); nc.sync.dma_start(b2c[:], d["b2col"][:])
        gbc = wpool.tile([NE, 1], F32); nc.sync.dma_start(gbc[:], d["gbcol"][:])
        e1bT = wpool.tile([128, NE], F32); nc.sync.dma_start(e1bT[:], d["e1bT"][:])
        e2bT = wpool.tile([128, NE], F32); nc.sync.dma_start(e2bT[:], d["e2bT"][:])
        smw = wpool.tile([128, 10], BF16); nc.sync.dma_start(smw[:], d["smw"][:])
        smb5 = wpool.tile([1, 50], BF16); nc.sync.dma_start(smb5[:], d["smb5"][:])
        ident = wpool.tile([128, 128], F32)
        make_identity(nc, ident[:])
        ones = wpool.tile([1, 128], BF16)
        nc.scalar.activation(ones[:], e2w[0:1, 0:128], AF.Copy, scale=0.0, bias=1.0)

        loop_cm = tc.For_i(0, loop_reps, 1) if loop_reps else contextlib.nullcontext()
        with loop_cm:
         for ch in range(NCH):
            b0 = ch * NB
            tA0 = xtp.tile([100, 12, NB], BF16, tag="tA0", name="tA0")
            nc.sync.dma_start(tA0[:], d["xa"][:, ch, 0:12])
            tB0 = xtp.tile([60, 12, NB], BF16, tag="tB0", name="tB0")
            nc.sync.dma_start(tB0[:], d["xb"][:, ch, 0:12])
            tA1 = xtp.tile([100, 12, NB], BF16, tag="tA1", name="tA1")
            nc.sync.dma_start(tA1[:], d["xa"][:, ch, 12:24])
            tB1 = xtp.tile([60, 12, NB], BF16, tag="tB1", name="tB1")
            nc.sync.dma_start(tB1[:], d["xb"][:, ch, 12:24])

            pp0, pp1, htiles = [], [], []

            def conv2_block(P):
                y0 = 2 * P
                for yy in (y0, y0 + 1):
                    for mci in range(2):
                        ps = c2ps.tile([128, NB], F32, tag="ps", name="c2t")
                        for r in range(3):
                            nc.tensor.matmul(
                                ps[:], w2p0[:, 256 * r + 128 * mci:256 * r + 128 * mci + 128],
                                pp0[yy + r][:], start=(r == 0),
                                stop=(mci == 0 and r == 2))
                        if mci == 1:
                            for r in range(3):
                                nc.tensor.matmul(ps[:], w2p1a[:, 128 * r:128 * r + 128],
                                                 pp1[yy + r][:], start=False,
                                                 stop=(r == 2))
                        h = hpool.tile([128, NB], BF16, tag="h", name="ht")
                        if mci == 1:
                            nc.vector.tensor_scalar(h[:], ps[:], b2c[:, 0:1], 0.0,
                                                    op0=ALU.add, op1=ALU.max)
                        else:
                            nc.scalar.activation(h[:], ps[:], AF.Relu, bias=b2c[:, 0:1])
                        htiles.append(h)
                ps = c2ps.tile([128, NB], F32, tag="ps", name="c2t")
                for rr in range(4):
                    nc.tensor.matmul(ps[:], w2p1b[:, 128 * rr:128 * rr + 128],
                                     pp1[y0 + rr][:], start=(rr == 0), stop=(rr == 3))
                h = hpool.tile([128, NB], BF16, tag="h", name="ht")
                nc.scalar.activation(h[:], ps[:], AF.Relu, bias=b2c[:, 0:1])
                htiles.append(h)

            # ---- conv1 (one matmul per tile) + pool, conv2 interleaved ----
            for Y in range(12):
                y0, y1 = 2 * Y, 2 * Y + 1
                tA = tA0 if y0 < 12 else tA1
                tB = tB0 if y0 < 12 else tB1
                yo0, yo1 = y0 % 12, y1 % 12
                tms = []
                for wsl, dat, tg in ((w1a[:, 0:128], tA, "tm0"),
                                     (w1a[:, 128:256], tA, "tm2"),
                                     (w1b[:, 0:128], tB, "tm1")):
                    pa = c1ps.tile([128, NB], F32, tag="ps", name="c1t")
                    nc.tensor.matmul(pa[:], wsl, dat[:, yo0, :], start=True, stop=True)
                    pb = c1ps.tile([128, NB], F32, tag="ps", name="c1t")
                    nc.tensor.matmul(pb[:], wsl, dat[:, yo1, :], start=True, stop=True)
                    ra = tmp.tile([128, NB], BF16, tag=tg + "r", name="rat")
                    nc.scalar.activation(ra[:], pa[:], AF.Copy)
                    tm = tmp.tile([128, NB], BF16, tag=tg, name="tmt")
                    nc.vector.tensor_tensor(tm[:], pb[:], ra[:], op=ALU.max)
                    tms.append(tm)
                tm0, tm2, tm1 = tms
                p0 = tmp.tile([128, NB], BF16, tag="p0", name="p0t")
                nc.vector.tensor_tensor(p0[:], tm0[:], tm2[:], op=ALU.max)
                sh = shp.tile([64, NB], BF16, tag="sh", name="sht")
                nc.sync.dma_start(sh[:], tm1[64:128, :])
                p1 = tmp.tile([64, NB], BF16, tag="p1", name="p1t")
                nc.vector.tensor_tensor(p1[:], tm1[0:64, :], sh[:], op=ALU.max)
                t0 = pp0p.tile([128, NB], BF16, tag="pp0", name="pp0t")
                nc.vector.tensor_scalar(t0[:], p0[:], b1c[:, 0:1], 0.0,
                                        op0=ALU.add, op1=ALU.max)
                t1 = pp1p.tile([64, NB], BF16, tag="pp1", name="pp1t")
                nc.vector.tensor_scalar(t1[:], p1[:], b1c[0:64, 0:1], 0.0,
                                        op0=ALU.add, op1=ALU.max)
                pp0.append(t0)
                pp1.append(t1)
                if Y >= 3 and Y % 2 == 1:
                    conv2_block((Y - 3) // 2)

            # ---- gate ----
            gps = mps.tile([NE, NB], F32, tag="m", name="gpst")
            for kc in range(25):
                nc.tensor.matmul(gps[:], gw[:, NE * kc:NE * kc + NE], htiles[kc][:],
                                 start=(kc == 0), stop=(kc == 24))
            gsb = gp.tile([NE, NB], F32, tag="gsb")
            nc.scalar.activation(gsb[:], gps[:], AF.Identity, bias=gbc[:, 0:1])

            # ---- experts (h2 of expert e-1 interleaved under e's h1) ----
            h1s = [None] * NE
            h2t = [None] * NE

            def h2_for(e):
                h2ps = mps.tile([128, NB], F32, tag="m", name="h2pst")
                nc.tensor.matmul(h2ps[:], e2w[:, DH * e:DH * e + DH], h1s[e][:],
                                 start=True, stop=True)
                h2 = h2p.tile([128, NB], BF16, tag="h2", name="h2t_")
                nc.scalar.activation(h2[:], h2ps[:], AF.Tanh, bias=e2bT[:, e:e + 1])
                h2t[e] = h2

            for e in range(NE):
                h1ps = mps.tile([128, NB], F32, tag="m", name="h1pst")
                for kc in range(25):
                    nc.tensor.matmul(h1ps[:], e1wt[e][:, DH * kc:DH * kc + DH],
                                     htiles[kc][:], start=(kc == 0), stop=(kc == 24))
                h1 = h1p.tile([128, NB], BF16, tag="h1", name="h1t_")
                nc.scalar.activation(h1[:], h1ps[:], AF.Tanh, bias=e1bT[:, e:e + 1])
                h1s[e] = h1
                if e >= 1:
                    h2_for(e - 1)
            h2_for(NE - 1)

            # ---- per-token-chunk: gating weights, head, softmax ----
            for t4 in range(NB // 128):
                tok = slice(128 * t4, 128 * t4 + 128)
                gtp = mps.tile([128, NE], F32, tag="m", name="gtpt")
                nc.tensor.transpose(gtp[:], gsb[:, tok], ident[0:NE, 0:NE])
                s = smallp.tile([128, NE], F32, tag="s")
                nc.scalar.activation(s[:], gtp[:], AF.Copy)
                mx = smallp.tile([128, 1], F32, tag="mx")
                nc.vector.reduce_max(mx[:], s[:], axis=AX.X)
                nmx = smallp.tile([128, 1], F32, tag="nmx")
                nc.vector.tensor_scalar_mul(nmx[:], mx[:], -1.0)
                ex = smallp.tile([128, NE], F32, tag="ex")
                nc.scalar.activation(ex[:], s[:], AF.Exp, bias=nmx[:, 0:1])
                gt = smallp.tile([128, NE * NE], F32, tag="gt")
                a_b = ex[:].unsqueeze(1).broadcast_to([128, NE, NE])
                b_b = ex[:].unsqueeze(2).broadcast_to([128, NE, NE])
                nc.vector.tensor_tensor(gt[:].rearrange("p (i j) -> p i j", i=NE),
                                        a_b, b_b, op=ALU.is_gt)
                rank = smallp.tile([128, NE], F32, tag="rank")
                nc.vector.reduce_sum(rank[:], gt[:].rearrange("p (i j) -> p i j", i=NE),
                                     axis=AX.X)
                m01 = smallp.tile([128, NE], F32, tag="m01")
                nc.vector.tensor_scalar(m01[:], rank[:], float(TOPK) - 0.5, None,
                                        op0=ALU.is_le)
                wun = smallp.tile([128, NE], F32, tag="wun")
                nc.vector.tensor_mul(wun[:], ex[:], m01[:])
                ssum = smallp.tile([128, 1], F32, tag="ssum")
                nc.vector.reduce_sum(ssum[:], wun[:], axis=AX.X)
                rinv = smallp.tile([128, 1], F32, tag="rinv")
                nc.vector.reciprocal(rinv[:], ssum[:])
                wfin = smallp.tile([128, NE], F32, tag="wfin")
                nc.vector.tensor_scalar_mul(wfin[:], wun[:], rinv[:, 0:1])

                lep = mps.tile([128, 50], F32, tag="m", name="lept")
                nc.tensor.matmul(lep[:, 0:50], ones[:], smb5[:], start=True, stop=False)
                for e in range(NE):
                    nc.tensor.matmul(lep[:, 10 * e:10 * e + 10], h2t[e][:, tok],
                                     smw[:], start=False, stop=(e == NE - 1))
                scl = smallp.tile([128, 50], F32, tag="scl")
                for e in range(NE):
                    nc.vector.tensor_scalar_mul(scl[:, 10 * e:10 * e + 10],
                                                lep[:, 10 * e:10 * e + 10],
                                                wfin[:, e:e + 1])
                logit = smallp.tile([128, 10], F32, tag="logit")
                nc.vector.reduce_sum(logit[:], scl[:].rearrange("p (e k) -> p k e", e=NE),
                                     axis=AX.X)
                lmx = smallp.tile([128, 1], F32, tag="lmx")
                nc.vector.reduce_max(lmx[:], logit[:], axis=AX.X)
                nlmx = smallp.tile([128, 1], F32, tag="nlmx")
                nc.vector.tensor_scalar_mul(nlmx[:], lmx[:], -1.0)
                lex = smallp.tile([128, 10], F32, tag="lex")
                nc.scalar.activation(lex[:], logit[:], AF.Exp, bias=nlmx[:, 0:1])
                lsum = smallp.tile([128, 1], F32, tag="lsum")
                nc.vector.reduce_sum(lsum[:], lex[:], axis=AX.X)
                lrinv = smallp.tile([128, 1], F32, tag="lrinv")
                nc.vector.reciprocal(lrinv[:], lsum[:])
                prob = smallp.tile([128, 10], F32, tag="prob")
                nc.vector.tensor_scalar_mul(prob[:], lex[:], lrinv[:, 0:1])
                nc.sync.dma_start(out_d[b0 + 128 * t4:b0 + 128 * t4 + 128, :], prob[:])


_NC_CACHE = None


def _get_nc():
    global _NC_CACHE
    if _NC_CACHE is None:
        _NC_CACHE = _build_nc()
    return _NC_CACHE


def kernel(**inputs):
    percore, w = _host_prep(**inputs)
    in_maps = [{**percore[c], **w} for c in range(NCORES)]
    nc = _get_nc()
    trace = bool(int(os.environ.get("KERNEL_TRACE", "0")))
    res = run_bass_kernel_spmd(nc, in_maps, list(range(NCORES)), trace=trace)
    kernel.last_results = res
    out = np.concatenate([res.results[c]["out"] for c in range(NCORES)], axis=0)
    return out.astype(np.float32)


# revision 19
# speedup vs baseline: 2.0611x; 1.0118x over previous
"""Trainium2 Bass kernel for nn_ClassifyModelMOE (conv feature extractor +
top-3-of-5 MoE + softmax head). Data-parallel over batch across 8 cores.

Self-contained: hardcodes all shapes; builds footprint-packed conv1 inputs and
Toeplitz-expanded conv weights on the host; runs one SPMD Bass/Tile program on
cores 0-7 via run_bass_kernel_spmd.

Structure per 512-token chunk:
  - 2 batched DMAs bring in footprint-packed x tiles (tA [100,24,512],
    tB [60,24,512]); conv1 is ONE matmul per output tile (K=95..100).
  - max-pool reads conv1 PSUM pairs directly (DVE), horizontal max on GpSimd,
    relu+bias fused into a DVE tensor_scalar on the pooled data.
  - conv2 blocks are interleaved into the conv1 row loop to keep PE fed.
  - expert-1 weights are SBUF-resident (loaded once).
"""
import os
import sys
import contextlib

sys.path.insert(0, "/opt/trn_rl_repo")

import numpy as np
import ml_dtypes

import concourse.bacc as bacc
import concourse.mybir as mybir
import concourse.tile as tile
from concourse.bass_utils import run_bass_kernel_spmd
from concourse.masks import make_identity

F32 = mybir.dt.float32
BF16 = mybir.dt.bfloat16
AF = mybir.ActivationFunctionType
ALU = mybir.AluOpType
AX = mybir.AxisListType

NCORES = 8
B = 8192
BC = B // NCORES          # tokens per core
NB = 512                  # batch chunk (columns per matmul)
NCH = BC // NB            # chunks per core
NE, TOPK = 5, 3
DH = 128

# conv1 output geometry: 16ch x 24x24; M-layout (per output row y):
#   Mc0: even x = 2j, j=0..7   -> m = j*16 + o        (128)
#   Mc2: odd x = 2j+1, j=0..7  -> m = j*16 + o        (128)
#   Mc1: x=16..23: [even j=8..11 | odd j=8..11]       (64+64)
# conv1 K-packing (footprint tiles):
#   tA part p = r*20 + c  <-> x[y+r, c],    c=0..19  (serves Mc0 and Mc2)
#   tB part p = r*12 + cc <-> x[y+r, 16+cc], cc=0..11 (serves Mc1)
# pooled row tiles: pp0 = j 0..7 (128 parts: j*16+c), pp1 = j 8..11 (64 parts)
# conv2 output (per row y): M = xout*32 + o2:
#   Mc0: xout 0..3 (128), Mc1: xout 4..7 (128),
#   Mc2pair: [y0: xout 8..9 | y1: xout 8..9] (64+64)


def _host_prep(x, conv1_w, conv1_b, conv2_w, conv2_b, gate_w, gate_b,
               e1_w, e1_b, e2_w, e2_b, sm_w, sm_b):
    bf = ml_dtypes.bfloat16
    x = np.asarray(x, np.float32)
    conv1_w = np.asarray(conv1_w, np.float32)
    conv2_w = np.asarray(conv2_w, np.float32)
    gate_w = np.asarray(gate_w, np.float32)
    e1_w = np.asarray(e1_w, np.float32)
    e2_w = np.asarray(e2_w, np.float32)

    xr = x.reshape(B, 28, 28)

    # footprint-packed x for conv1 (host-duplicated, per-core sliced below)
    xa_full = np.empty((100, B, 24), np.float32)
    for r in range(5):
        for c in range(20):
            xa_full[r * 20 + c] = xr[:, r:r + 24, c]
    xb_full = np.empty((60, B, 24), np.float32)
    for r in range(5):
        for cc in range(12):
            xb_full[r * 12 + cc] = xr[:, r:r + 24, 16 + cc]
    xa_full = xa_full.astype(bf)
    xb_full = xb_full.astype(bf)

    percore = []
    for cid in range(NCORES):
        c0 = cid * BC
        xa = xa_full[:, c0:c0 + BC, :].reshape(100, NCH, NB, 24)
        xa = np.ascontiguousarray(xa.transpose(0, 1, 3, 2))   # [100,NCH,24,NB]
        xb = xb_full[:, c0:c0 + BC, :].reshape(60, NCH, NB, 24)
        xb = np.ascontiguousarray(xb.transpose(0, 1, 3, 2))
        percore.append({"xa": xa, "xb": xb})

    # conv1 stationaries for the footprint tiles
    w1a = np.zeros((100, 256), np.float32)
    w1b = np.zeros((60, 128), np.float32)
    for r in range(5):
        for dx in range(5):
            for j in range(8):
                for o in range(16):
                    w1a[r * 20 + 2 * j + dx, j * 16 + o] = conv1_w[o, 0, r, dx]
                    w1a[r * 20 + 2 * j + 1 + dx, 128 + j * 16 + o] = conv1_w[o, 0, r, dx]
            for j in range(8, 12):
                for o in range(16):
                    w1b[r * 12 + 2 * (j - 8) + dx, (j - 8) * 16 + o] = conv1_w[o, 0, r, dx]
                    w1b[r * 12 + 2 * (j - 8) + 1 + dx, 64 + (j - 8) * 16 + o] = conv1_w[o, 0, r, dx]

    # conv2 toeplitz: pooled row layout p = j*16 + c (pp0: j<8), (j-8)*16+c (pp1)
    # w2p0 [3, 128, 256]: r taps, cols [Mc0 | Mc1]
    w2p0 = np.zeros((3, 128, 256), np.float32)
    w2p1mc1 = np.zeros((3, 64, 128), np.float32)
    w2p1mc2 = np.zeros((4, 64, 128), np.float32)
    for r in range(3):
        for j in range(8):
            for c in range(16):
                p = j * 16 + c
                for mci, xobase in ((0, 0), (1, 4)):
                    for xo in range(xobase, xobase + 4):
                        dx = j - xo
                        if 0 <= dx < 3:
                            for o2 in range(32):
                                w2p0[r, p, 128 * mci + (xo - xobase) * 32 + o2] = \
                                    conv2_w[o2, c, r, dx]
        for j in range(8, 12):
            for c in range(16):
                p = (j - 8) * 16 + c
                for xo in range(4, 8):
                    dx = j - xo
                    if 0 <= dx < 3:
                        for o2 in range(32):
                            w2p1mc1[r, p, (xo - 4) * 32 + o2] = conv2_w[o2, c, r, dx]
    for rr in range(4):
        for b_ in range(2):
            r = rr - b_
            if not (0 <= r < 3):
                continue
            for j in range(8, 12):
                for c in range(16):
                    p = (j - 8) * 16 + c
                    for xo in range(8, 10):
                        dx = j - xo
                        if 0 <= dx < 3:
                            for o2 in range(32):
                                w2p1mc2[rr, p, 64 * b_ + (xo - 8) * 32 + o2] = \
                                    conv2_w[o2, c, r, dx]

    # h feature permutation: our flat index (tile*128+part) -> reference f = o2*100 + y*10 + xo
    perm = np.zeros(3200, np.int64)
    for P in range(5):
        y0, y1 = 2 * P, 2 * P + 1
        tiles = []
        for yy in (y0, y1):
            for xobase in (0, 4):
                tiles.append([(yy, xo, o2) for xo in range(xobase, xobase + 4)
                              for o2 in range(32)])
        t4 = [(y0, xo, o2) for xo in range(8, 10) for o2 in range(32)] + \
             [(y1, xo, o2) for xo in range(8, 10) for o2 in range(32)]
        order = [tiles[0], tiles[1], tiles[2], tiles[3], t4]
        for ti, tl in enumerate(order):
            for p, (yy, xo, o2) in enumerate(tl):
                perm[(5 * P + ti) * 128 + p] = o2 * 100 + yy * 10 + xo
    e1wpk = e1_w[:, perm, :].reshape(NE, 25, 128, DH)          # [E,kc,128,DH]
    e1wr = np.ascontiguousarray(e1wpk.transpose(0, 2, 1, 3)).reshape(NE, 128, 25 * DH)
    gwp = gate_w[perm, :].reshape(25, 128, NE).astype(np.float32)

    b1col = np.asarray(conv1_b, np.float32)[np.arange(128) % 16].reshape(128, 1)
    b2col = np.asarray(conv2_b, np.float32)[np.arange(128) % 32].reshape(128, 1)
    gbcol = np.asarray(gate_b, np.float32).reshape(NE, 1)
    e1bT = np.asarray(e1_b, np.float32).T.copy()      # [128, 5]
    e2bT = np.asarray(e2_b, np.float32).T.copy()      # [128, 5]
    smw = np.asarray(sm_w, np.float32)                # [128, 10]
    smb5 = np.tile(np.asarray(sm_b, np.float32), 5).reshape(1, 50)

    weights = dict(
        w1a=w1a.astype(bf), w1b=w1b.astype(bf),
        w2p0=np.ascontiguousarray(w2p0.transpose(1, 0, 2)).reshape(128, 768).astype(bf),
        w2p1mc1=np.ascontiguousarray(w2p1mc1.transpose(1, 0, 2)).reshape(64, 384).astype(bf),
        w2p1mc2=np.ascontiguousarray(w2p1mc2.transpose(1, 0, 2)).reshape(64, 512).astype(bf),
        e1wr=e1wr.astype(bf),
        gwp=np.ascontiguousarray(gwp.transpose(1, 0, 2)).reshape(128, 125).astype(bf),
        e2w=np.ascontiguousarray(e2_w.astype(np.float32).transpose(1, 0, 2)).reshape(128, 640).astype(bf),
        b1col=b1col, b2col=b2col,
        gbcol=gbcol, e1bT=e1bT, e2bT=e2bT, smw=smw.astype(bf), smb5=smb5.astype(bf))
    return percore, weights


def _build_nc(loop_reps=None, body_mult=1):
    nc = bacc.Bacc("TRN2", target_bir_lowering=False, debug=False)
    d = {}
    def din(name, shape, dt):
        d[name] = nc.dram_tensor(name, list(shape), dt, kind="ExternalInput").ap()
    din("xa", (100, NCH, 24, NB), BF16)
    din("xb", (60, NCH, 24, NB), BF16)
    din("w1a", (100, 256), BF16)
    din("w1b", (60, 128), BF16)
    din("w2p0", (128, 768), BF16)
    din("w2p1mc1", (64, 384), BF16)
    din("w2p1mc2", (64, 512), BF16)
    din("e1wr", (NE, 128, 25 * DH), BF16)
    din("gwp", (128, 125), BF16)
    din("e2w", (128, 640), BF16)
    din("b1col", (128, 1), F32)
    din("b2col", (128, 1), F32)
    din("gbcol", (NE, 1), F32)
    din("e1bT", (128, NE), F32)
    din("e2bT", (128, NE), F32)
    din("smw", (128, 10), BF16)
    din("smb5", (1, 50), BF16)
    out_d = nc.dram_tensor("out", [BC, 10], F32, kind="ExternalOutput").ap()

    with tile.TileContext(nc) as tc:
        _emit(nc, tc, d, out_d, loop_reps=loop_reps, body_mult=body_mult)
    nc.compile()
    return nc


def _emit(nc, tc, d, out_d, loop_reps=None, body_mult=1):
    ctx = contextlib.ExitStack()
    with ctx:
        wpool = ctx.enter_context(tc.tile_pool(name="wpool", bufs=1))
        xtp = ctx.enter_context(tc.tile_pool(name="xtp", bufs=1))
        tmp = ctx.enter_context(tc.tile_pool(name="tmp", bufs=2))
        shp = ctx.enter_context(tc.tile_pool(name="shp", bufs=2))
        pp0p = ctx.enter_context(tc.tile_pool(name="pp0p", bufs=12))
        pp1p = ctx.enter_context(tc.tile_pool(name="pp1p", bufs=12))
        hpool = ctx.enter_context(tc.tile_pool(name="hpool", bufs=25))
        h1p = ctx.enter_context(tc.tile_pool(name="h1p", bufs=2))
        h2p = ctx.enter_context(tc.tile_pool(name="h2p", bufs=5))
        gp = ctx.enter_context(tc.tile_pool(name="gp", bufs=2))
        smallp = ctx.enter_context(tc.tile_pool(name="smallp", bufs=8))
        c1ps = ctx.enter_context(tc.tile_pool(name="c1ps", bufs=4, space="PSUM"))
        c2ps = ctx.enter_context(tc.tile_pool(name="c2ps", bufs=2, space="PSUM"))
        mps = ctx.enter_context(tc.tile_pool(name="mps", bufs=2, space="PSUM"))

        # resident weights
        w1a = wpool.tile([100, 256], BF16); nc.sync.dma_start(w1a[:], d["w1a"][:])
        w1b = wpool.tile([60, 128], BF16); nc.sync.dma_start(w1b[:], d["w1b"][:])
        w2p0 = wpool.tile([128, 3 * 256], BF16)
        nc.sync.dma_start(w2p0[:], d["w2p0"][:])
        w2p1a = wpool.tile([64, 3 * 128], BF16)
        nc.sync.dma_start(w2p1a[:], d["w2p1mc1"][:])
        w2p1b = wpool.tile([64, 4 * 128], BF16)
        nc.sync.dma_start(w2p1b[:], d["w2p1mc2"][:])
        gw = wpool.tile([128, 25 * NE], BF16)
        nc.sync.dma_start(gw[:], d["gwp"][:])
        e2w = wpool.tile([128, NE * DH], BF16)
        nc.sync.dma_start(e2w[:], d["e2w"][:])
        e1wt = []
        for e in range(NE):
            t = wpool.tile([128, 25 * DH], BF16, tag=f"e1w{e}", name=f"e1w{e}")
            nc.sync.dma_start(t[:], d["e1wr"][e])
            e1wt.append(t)
        b1c = wpool.tile([128, 1], F32); nc.sync.dma_start(b1c[:], d["b1col"][:])
        b2c = wpool.tile([128, 1], F32); nc.sync.dma_start(b2c[:], d["b2col"][:])
        gbc = wpool.tile([NE, 1], F32); nc.sync.dma_start(gbc[:], d["gbcol"][:])
        e1bT = wpool.tile([128, NE], F32); nc.sync.dma_start(e1bT[:], d["e1bT"][:])
        e2bT = wpool.tile([128, NE], F32); nc.sync.dma_start(e2bT[:], d["e2bT"][:])
        smw = wpool.tile([128, 10], BF16); nc.sync.dma_start(smw[:], d["smw"][:])
        smb5 = wpool.tile([1, 50], BF16); nc.sync.dma_start(smb5[:], d["smb5"][:])
        ident = wpool.tile([128, 128], F32)
        make_identity(nc, ident[:])
        ones = wpool.tile([1, 128], BF16)
        nc.scalar.activation(ones[:], e2w[0:1, 0:128], AF.Copy, scale=0.0, bias=1.0)

        loop_cm = tc.For_i(0, loop_reps, 1) if loop_reps else contextlib.nullcontext()
        with loop_cm:
         for ch in [c % NCH for c in range(NCH * body_mult)]:
            b0 = ch * NB
            tA0 = xtp.tile([100, 12, NB], BF16, tag="tA0", name="tA0")
            nc.sync.dma_start(tA0[:], d["xa"][:, ch, 0:12])
            tB0 = xtp.tile([60, 12, NB], BF16, tag="tB0", name="tB0")
            nc.sync.dma_start(tB0[:], d["xb"][:, ch, 0:12])
            tA1 = xtp.tile([100, 12, NB], BF16, tag="tA1", name="tA1")
            nc.sync.dma_start(tA1[:], d["xa"][:, ch, 12:24])
            tB1 = xtp.tile([60, 12, NB], BF16, tag="tB1", name="tB1")
            nc.sync.dma_start(tB1[:], d["xb"][:, ch, 12:24])

            pp0, pp1, htiles = [], [], []

            def conv2_block(P):
                y0 = 2 * P
                for yy in (y0, y0 + 1):
                    for mci in range(2):
                        ps = c2ps.tile([128, NB], F32, tag="ps", name="c2t")
                        for r in range(3):
                            nc.tensor.matmul(
                                ps[:], w2p0[:, 256 * r + 128 * mci:256 * r + 128 * mci + 128],
                                pp0[yy + r][:], start=(r == 0),
                                stop=(mci == 0 and r == 2))
                        if mci == 1:
                            for r in range(3):
                                nc.tensor.matmul(ps[:], w2p1a[:, 128 * r:128 * r + 128],
                                                 pp1[yy + r][:], start=False,
                                                 stop=(r == 2))
                        h = hpool.tile([128, NB], BF16, tag="h", name="ht")
                        nc.scalar.activation(h[:], ps[:], AF.Relu, bias=b2c[:, 0:1])
                        htiles.append(h)
                ps = c2ps.tile([128, NB], F32, tag="ps", name="c2t")
                for rr in range(4):
                    nc.tensor.matmul(ps[:], w2p1b[:, 128 * rr:128 * rr + 128],
                                     pp1[y0 + rr][:], start=(rr == 0), stop=(rr == 3))
                h = hpool.tile([128, NB], BF16, tag="h", name="ht")
                nc.scalar.activation(h[:], ps[:], AF.Relu, bias=b2c[:, 0:1])
                htiles.append(h)

            # ---- conv1 (one matmul per tile) + pool, conv2 interleaved ----
            for Y in range(12):
                y0, y1 = 2 * Y, 2 * Y + 1
                tA = tA0 if y0 < 12 else tA1
                tB = tB0 if y0 < 12 else tB1
                yo0, yo1 = y0 % 12, y1 % 12
                tms = []
                for wsl, dat, tg in ((w1a[:, 0:128], tA, "tm0"),
                                     (w1a[:, 128:256], tA, "tm2"),
                                     (w1b[:, 0:128], tB, "tm1")):
                    pa = c1ps.tile([128, NB], F32, tag="ps", name="c1t")
                    nc.tensor.matmul(pa[:], wsl, dat[:, yo0, :], start=True, stop=True)
                    pb = c1ps.tile([128, NB], F32, tag="ps", name="c1t")
                    nc.tensor.matmul(pb[:], wsl, dat[:, yo1, :], start=True, stop=True)
                    ra = tmp.tile([128, NB], BF16, tag=tg + "r", name="rat")
                    nc.scalar.activation(ra[:], pa[:], AF.Copy)
                    tm = tmp.tile([128, NB], BF16, tag=tg, name="tmt")
                    nc.vector.tensor_tensor(tm[:], pb[:], ra[:], op=ALU.max)
                    tms.append(tm)
                tm0, tm2, tm1 = tms
                p0 = tmp.tile([128, NB], BF16, tag="p0", name="p0t")
                nc.vector.tensor_tensor(p0[:], tm0[:], tm2[:], op=ALU.max)
                sh = shp.tile([64, NB], BF16, tag="sh", name="sht")
                nc.sync.dma_start(sh[:], tm1[64:128, :])
                p1 = tmp.tile([64, NB], BF16, tag="p1", name="p1t")
                nc.vector.tensor_tensor(p1[:], tm1[0:64, :], sh[:], op=ALU.max)
                t0 = pp0p.tile([128, NB], BF16, tag="pp0", name="pp0t")
                nc.vector.tensor_scalar(t0[:], p0[:], b1c[:, 0:1], 0.0,
                                        op0=ALU.add, op1=ALU.max)
                t1 = pp1p.tile([64, NB], BF16, tag="pp1", name="pp1t")
                nc.vector.tensor_scalar(t1[:], p1[:], b1c[0:64, 0:1], 0.0,
                                        op0=ALU.add, op1=ALU.max)
                pp0.append(t0)
                pp1.append(t1)
                if Y >= 3 and Y % 2 == 1:
                    conv2_block((Y - 3) // 2)

            # ---- gate ----
            gps = mps.tile([NE, NB], F32, tag="m", name="gpst")
            for kc in range(25):
                nc.tensor.matmul(gps[:], gw[:, NE * kc:NE * kc + NE], htiles[kc][:],
                                 start=(kc == 0), stop=(kc == 24))
            gsb = gp.tile([NE, NB], F32, tag="gsb")
            nc.scalar.activation(gsb[:], gps[:], AF.Identity, bias=gbc[:, 0:1])

            # ---- experts (h2 of expert e-1 interleaved under e's h1) ----
            h1s = [None] * NE
            h2t = [None] * NE

            def h2_for(e):
                h2ps = mps.tile([128, NB], F32, tag="m", name="h2pst")
                nc.tensor.matmul(h2ps[:], e2w[:, DH * e:DH * e + DH], h1s[e][:],
                                 start=True, stop=True)
                h2 = h2p.tile([128, NB], BF16, tag="h2", name="h2t_")
                nc.scalar.activation(h2[:], h2ps[:], AF.Tanh, bias=e2bT[:, e:e + 1])
                h2t[e] = h2

            for e in range(NE):
                h1ps = mps.tile([128, NB], F32, tag="m", name="h1pst")
                for kc in range(25):
                    nc.tensor.matmul(h1ps[:], e1wt[e][:, DH * kc:DH * kc + DH],
                                     htiles[kc][:], start=(kc == 0), stop=(kc == 24))
                h1 = h1p.tile([128, NB], BF16, tag="h1", name="h1t_")
                nc.scalar.activation(h1[:], h1ps[:], AF.Tanh, bias=e1bT[:, e:e + 1])
                h1s[e] = h1
                if e >= 1:
                    h2_for(e - 1)
            h2_for(NE - 1)

            # ---- per-token-chunk: gating weights, head, softmax ----
            for t4 in range(NB // 128):
                tok = slice(128 * t4, 128 * t4 + 128)
                hd = mps.tile([128, 64], F32, tag="m", name="hdt")
                gtp = hd[:, 0:NE]
                nc.tensor.transpose(gtp, gsb[:, tok], ident[0:NE, 0:NE])
                s = smallp.tile([128, NE], F32, tag="s")
                nc.scalar.activation(s[:], gtp, AF.Copy)
                mx = smallp.tile([128, 1], F32, tag="mx")
                nc.vector.reduce_max(mx[:], s[:], axis=AX.X)
                nmx = smallp.tile([128, 1], F32, tag="nmx")
                nc.vector.tensor_scalar_mul(nmx[:], mx[:], -1.0)
                ex = smallp.tile([128, NE], F32, tag="ex")
                nc.scalar.activation(ex[:], s[:], AF.Exp, bias=nmx[:, 0:1])
                gt = smallp.tile([128, NE * NE], F32, tag="gt")
                a_b = ex[:].unsqueeze(1).broadcast_to([128, NE, NE])
                b_b = ex[:].unsqueeze(2).broadcast_to([128, NE, NE])
                nc.vector.tensor_tensor(gt[:].rearrange("p (i j) -> p i j", i=NE),
                                        a_b, b_b, op=ALU.is_gt)
                rank = smallp.tile([128, NE], F32, tag="rank")
                nc.vector.reduce_sum(rank[:], gt[:].rearrange("p (i j) -> p i j", i=NE),
                                     axis=AX.X)
                m01 = smallp.tile([128, NE], F32, tag="m01")
                nc.vector.tensor_scalar(m01[:], rank[:], float(TOPK) - 0.5, None,
                                        op0=ALU.is_le)
                wun = smallp.tile([128, NE], F32, tag="wun")
                nc.vector.tensor_mul(wun[:], ex[:], m01[:])
                ssum = smallp.tile([128, 1], F32, tag="ssum")
                nc.vector.reduce_sum(ssum[:], wun[:], axis=AX.X)
                rinv = smallp.tile([128, 1], F32, tag="rinv")
                nc.vector.reciprocal(rinv[:], ssum[:])
                wfin = smallp.tile([128, NE], F32, tag="wfin")
                nc.vector.tensor_scalar_mul(wfin[:], wun[:], rinv[:, 0:1])

                lep = hd[:, 8:58]
                nc.tensor.matmul(lep[:, 0:50], ones[:], smb5[:], start=True, stop=False)
                for e in range(NE):
                    nc.tensor.matmul(lep[:, 10 * e:10 * e + 10], h2t[e][:, tok],
                                     smw[:], start=False, stop=(e == NE - 1))
                scl = smallp.tile([128, 50], F32, tag="scl")
                for e in range(NE):
                    nc.vector.tensor_scalar_mul(scl[:, 10 * e:10 * e + 10],
                                                lep[:, 10 * e:10 * e + 10],
                                                wfin[:, e:e + 1])
                logit = smallp.tile([128, 10], F32, tag="logit")
                nc.vector.reduce_sum(logit[:], scl[:].rearrange("p (e k) -> p k e", e=NE),
                                     axis=AX.X)
                lmx = smallp.tile([128, 1], F32, tag="lmx")
                nc.vector.reduce_max(lmx[:], logit[:], axis=AX.X)
                nlmx = smallp.tile([128, 1], F32, tag="nlmx")
                nc.vector.tensor_scalar_mul(nlmx[:], lmx[:], -1.0)
                lex = smallp.tile([128, 10], F32, tag="lex")
                nc.scalar.activation(lex[:], logit[:], AF.Exp, bias=nlmx[:, 0:1])
                lsum = smallp.tile([128, 1], F32, tag="lsum")
                nc.vector.reduce_sum(lsum[:], lex[:], axis=AX.X)
                lrinv = smallp.tile([128, 1], F32, tag="lrinv")
                nc.vector.reciprocal(lrinv[:], lsum[:])
                prob = smallp.tile([128, 10], F32, tag="prob")
                nc.vector.tensor_scalar_mul(prob[:], lex[:], lrinv[:, 0:1])
                nc.sync.dma_start(out_d[b0 + 128 * t4:b0 + 128 * t4 + 128, :], prob[:])


_NC_CACHE = None


def _get_nc():
    global _NC_CACHE
    if _NC_CACHE is None:
        _NC_CACHE = _build_nc()
    return _NC_CACHE


def kernel(**inputs):
    percore, w = _host_prep(**inputs)
    in_maps = [{**percore[c], **w} for c in range(NCORES)]
    nc = _get_nc()
    trace = bool(int(os.environ.get("KERNEL_TRACE", "0")))
    res = run_bass_kernel_spmd(nc, in_maps, list(range(NCORES)), trace=trace)
    kernel.last_results = res
    out = np.concatenate([res.results[c]["out"] for c in range(NCORES)], axis=0)
    return out.astype(np.float32)


# revision 20
# speedup vs baseline: 2.2085x; 1.0715x over previous
"""Trainium2 Bass kernel for nn_ClassifyModelMOE (conv feature extractor +
top-3-of-5 MoE + softmax head). Data-parallel over batch across 8 cores.

Self-contained: hardcodes all shapes; builds footprint-packed conv1 inputs and
Toeplitz-expanded conv weights on the host; runs one SPMD Bass/Tile program on
cores 0-7 via run_bass_kernel_spmd.

Structure per 512-token chunk:
  - 2 batched DMAs bring in footprint-packed x tiles (tA [100,24,512],
    tB [60,24,512]); conv1 is ONE matmul per output tile (K=95..100).
  - max-pool reads conv1 PSUM pairs directly (DVE), horizontal max on GpSimd,
    relu+bias fused into a DVE tensor_scalar on the pooled data.
  - conv2 blocks are interleaved into the conv1 row loop to keep PE fed.
  - expert-1 weights are SBUF-resident (loaded once).
"""
import os
import sys
import contextlib

sys.path.insert(0, "/opt/trn_rl_repo")

import numpy as np
import ml_dtypes

import concourse.bacc as bacc
import concourse.mybir as mybir
import concourse.tile as tile
from concourse.bass_utils import run_bass_kernel_spmd
from concourse.masks import make_identity

F32 = mybir.dt.float32
BF16 = mybir.dt.bfloat16
AF = mybir.ActivationFunctionType
ALU = mybir.AluOpType
AX = mybir.AxisListType

NCORES = 8
B = 8192
BC = B // NCORES          # tokens per core
NB = 512                  # batch chunk (columns per matmul)
NCH = BC // NB            # chunks per core
NE, TOPK = 5, 3
DH = 128

# conv1 output geometry: 16ch x 24x24; M-layout (per output row y):
#   Mc0: even x = 2j, j=0..7   -> m = j*16 + o        (128)
#   Mc2: odd x = 2j+1, j=0..7  -> m = j*16 + o        (128)
#   Mc1: x=16..23: [even j=8..11 | odd j=8..11]       (64+64)
# conv1 K-packing (footprint tiles):
#   tA part p = r*20 + c  <-> x[y+r, c],    c=0..19  (serves Mc0 and Mc2)
#   tB part p = r*12 + cc <-> x[y+r, 16+cc], cc=0..11 (serves Mc1)
# pooled row tiles: pp0 = j 0..7 (128 parts: j*16+c), pp1 = j 8..11 (64 parts)
# conv2 output (per row y): M = xout*32 + o2:
#   Mc0: xout 0..3 (128), Mc1: xout 4..7 (128),
#   Mc2pair: [y0: xout 8..9 | y1: xout 8..9] (64+64)


def _host_prep(x, conv1_w, conv1_b, conv2_w, conv2_b, gate_w, gate_b,
               e1_w, e1_b, e2_w, e2_b, sm_w, sm_b):
    bf = ml_dtypes.bfloat16
    x = np.asarray(x, np.float32)
    conv1_w = np.asarray(conv1_w, np.float32)
    conv2_w = np.asarray(conv2_w, np.float32)
    gate_w = np.asarray(gate_w, np.float32)
    e1_w = np.asarray(e1_w, np.float32)
    e2_w = np.asarray(e2_w, np.float32)

    xr = x.reshape(B, 28, 28)

    # footprint-packed x for conv1 (host-duplicated, per-core sliced below)
    xa_full = np.empty((100, B, 24), np.float32)
    for r in range(5):
        for c in range(20):
            xa_full[r * 20 + c] = xr[:, r:r + 24, c]
    xb_full = np.empty((60, B, 24), np.float32)
    for r in range(5):
        for cc in range(12):
            xb_full[r * 12 + cc] = xr[:, r:r + 24, 16 + cc]
    xa_full = xa_full.astype(bf)
    xb_full = xb_full.astype(bf)

    percore = []
    for cid in range(NCORES):
        c0 = cid * BC
        xa = xa_full[:, c0:c0 + BC, :].reshape(100, NCH, NB, 24)
        xa = np.ascontiguousarray(xa.transpose(0, 1, 3, 2))   # [100,NCH,24,NB]
        xb = xb_full[:, c0:c0 + BC, :].reshape(60, NCH, NB, 24)
        xb = np.ascontiguousarray(xb.transpose(0, 1, 3, 2))
        percore.append({"xa": xa, "xb": xb})

    # conv1 stationaries for the footprint tiles
    w1a = np.zeros((100, 256), np.float32)
    w1b = np.zeros((60, 128), np.float32)
    for r in range(5):
        for dx in range(5):
            for j in range(8):
                for o in range(16):
                    w1a[r * 20 + 2 * j + dx, j * 16 + o] = conv1_w[o, 0, r, dx]
                    w1a[r * 20 + 2 * j + 1 + dx, 128 + j * 16 + o] = conv1_w[o, 0, r, dx]
            for j in range(8, 12):
                for o in range(16):
                    w1b[r * 12 + 2 * (j - 8) + dx, (j - 8) * 16 + o] = conv1_w[o, 0, r, dx]
                    w1b[r * 12 + 2 * (j - 8) + 1 + dx, 64 + (j - 8) * 16 + o] = conv1_w[o, 0, r, dx]

    # conv2 toeplitz: pooled row layout p = j*16 + c (pp0: j<8), (j-8)*16+c (pp1)
    # w2p0 [3, 128, 256]: r taps, cols [Mc0 | Mc1]
    w2p0 = np.zeros((3, 128, 256), np.float32)
    w2p1mc1 = np.zeros((3, 64, 128), np.float32)
    w2p1mc2 = np.zeros((4, 64, 128), np.float32)
    for r in range(3):
        for j in range(8):
            for c in range(16):
                p = j * 16 + c
                for mci, xobase in ((0, 0), (1, 4)):
                    for xo in range(xobase, xobase + 4):
                        dx = j - xo
                        if 0 <= dx < 3:
                            for o2 in range(32):
                                w2p0[r, p, 128 * mci + (xo - xobase) * 32 + o2] = \
                                    conv2_w[o2, c, r, dx]
        for j in range(8, 12):
            for c in range(16):
                p = (j - 8) * 16 + c
                for xo in range(4, 8):
                    dx = j - xo
                    if 0 <= dx < 3:
                        for o2 in range(32):
                            w2p1mc1[r, p, (xo - 4) * 32 + o2] = conv2_w[o2, c, r, dx]
    for rr in range(4):
        for b_ in range(2):
            r = rr - b_
            if not (0 <= r < 3):
                continue
            for j in range(8, 12):
                for c in range(16):
                    p = (j - 8) * 16 + c
                    for xo in range(8, 10):
                        dx = j - xo
                        if 0 <= dx < 3:
                            for o2 in range(32):
                                w2p1mc2[rr, p, 64 * b_ + (xo - 8) * 32 + o2] = \
                                    conv2_w[o2, c, r, dx]

    # h feature permutation: our flat index (tile*128+part) -> reference f = o2*100 + y*10 + xo
    perm = np.zeros(3200, np.int64)
    for P in range(5):
        y0, y1 = 2 * P, 2 * P + 1
        tiles = []
        for yy in (y0, y1):
            for xobase in (0, 4):
                tiles.append([(yy, xo, o2) for xo in range(xobase, xobase + 4)
                              for o2 in range(32)])
        t4 = [(y0, xo, o2) for xo in range(8, 10) for o2 in range(32)] + \
             [(y1, xo, o2) for xo in range(8, 10) for o2 in range(32)]
        order = [tiles[0], tiles[1], tiles[2], tiles[3], t4]
        for ti, tl in enumerate(order):
            for p, (yy, xo, o2) in enumerate(tl):
                perm[(5 * P + ti) * 128 + p] = o2 * 100 + yy * 10 + xo
    e1wpk = e1_w[:, perm, :].reshape(NE, 25, 128, DH)          # [E,kc,128,DH]
    e1wr = np.ascontiguousarray(e1wpk.transpose(0, 2, 1, 3)).reshape(NE, 128, 25 * DH)
    gwp = gate_w[perm, :].reshape(25, 128, NE).astype(np.float32)

    b1col = np.asarray(conv1_b, np.float32)[np.arange(128) % 16].reshape(128, 1)
    b2col = np.asarray(conv2_b, np.float32)[np.arange(128) % 32].reshape(128, 1)
    gbcol = np.asarray(gate_b, np.float32).reshape(NE, 1)
    e1bT = np.asarray(e1_b, np.float32).T.copy()      # [128, 5]
    e2bT = np.asarray(e2_b, np.float32).T.copy()      # [128, 5]
    smw = np.asarray(sm_w, np.float32)                # [128, 10]
    smb5 = np.tile(np.asarray(sm_b, np.float32), 5).reshape(1, 50)

    weights = dict(
        w1a=w1a.astype(bf), w1b=w1b.astype(bf),
        w2p0=np.ascontiguousarray(w2p0.transpose(1, 0, 2)).reshape(128, 768).astype(bf),
        w2p1mc1=np.ascontiguousarray(w2p1mc1.transpose(1, 0, 2)).reshape(64, 384).astype(bf),
        w2p1mc2=np.ascontiguousarray(w2p1mc2.transpose(1, 0, 2)).reshape(64, 512).astype(bf),
        e1wr=e1wr.astype(bf),
        gwp=np.ascontiguousarray(gwp.transpose(1, 0, 2)).reshape(128, 125).astype(bf),
        e2w=np.ascontiguousarray(e2_w.astype(np.float32).transpose(1, 0, 2)).reshape(128, 640).astype(bf),
        b1col=b1col, b2col=b2col,
        gbcol=gbcol, e1bT=e1bT, e2bT=e2bT, smw=smw.astype(bf), smb5=smb5.astype(bf))
    return percore, weights


def _build_nc(loop_reps=None, body_mult=1):
    nc = bacc.Bacc("TRN2", target_bir_lowering=False, debug=False)
    d = {}
    def din(name, shape, dt):
        d[name] = nc.dram_tensor(name, list(shape), dt, kind="ExternalInput").ap()
    din("xa", (100, NCH, 24, NB), BF16)
    din("xb", (60, NCH, 24, NB), BF16)
    din("w1a", (100, 256), BF16)
    din("w1b", (60, 128), BF16)
    din("w2p0", (128, 768), BF16)
    din("w2p1mc1", (64, 384), BF16)
    din("w2p1mc2", (64, 512), BF16)
    din("e1wr", (NE, 128, 25 * DH), BF16)
    din("gwp", (128, 125), BF16)
    din("e2w", (128, 640), BF16)
    din("b1col", (128, 1), F32)
    din("b2col", (128, 1), F32)
    din("gbcol", (NE, 1), F32)
    din("e1bT", (128, NE), F32)
    din("e2bT", (128, NE), F32)
    din("smw", (128, 10), BF16)
    din("smb5", (1, 50), BF16)
    out_d = nc.dram_tensor("out", [BC, 10], F32, kind="ExternalOutput").ap()

    with tile.TileContext(nc) as tc:
        _emit(nc, tc, d, out_d, loop_reps=loop_reps, body_mult=body_mult)
    nc.compile()
    return nc


def _emit(nc, tc, d, out_d, loop_reps=None, body_mult=1):
    ctx = contextlib.ExitStack()
    with ctx:
        wpool = ctx.enter_context(tc.tile_pool(name="wpool", bufs=1))
        xtp = ctx.enter_context(tc.tile_pool(name="xtp", bufs=1))
        tmp = ctx.enter_context(tc.tile_pool(name="tmp", bufs=2))
        shp = ctx.enter_context(tc.tile_pool(name="shp", bufs=2))
        pp0p = ctx.enter_context(tc.tile_pool(name="pp0p", bufs=12))
        pp1p = ctx.enter_context(tc.tile_pool(name="pp1p", bufs=12))
        hpool = ctx.enter_context(tc.tile_pool(name="hpool", bufs=25))
        h1p = ctx.enter_context(tc.tile_pool(name="h1p", bufs=2))
        h2p = ctx.enter_context(tc.tile_pool(name="h2p", bufs=5))
        gp = ctx.enter_context(tc.tile_pool(name="gp", bufs=2))
        smallp = ctx.enter_context(tc.tile_pool(name="smallp", bufs=8))
        c1ps = ctx.enter_context(tc.tile_pool(name="c1ps", bufs=4, space="PSUM"))
        c2ps = ctx.enter_context(tc.tile_pool(name="c2ps", bufs=2, space="PSUM"))
        mps = ctx.enter_context(tc.tile_pool(name="mps", bufs=2, space="PSUM"))

        # resident weights
        w1a = wpool.tile([100, 256], BF16); nc.sync.dma_start(w1a[:], d["w1a"][:])
        w1b = wpool.tile([60, 128], BF16); nc.sync.dma_start(w1b[:], d["w1b"][:])
        w2p0 = wpool.tile([128, 3 * 256], BF16)
        nc.sync.dma_start(w2p0[:], d["w2p0"][:])
        w2p1a = wpool.tile([64, 3 * 128], BF16)
        nc.sync.dma_start(w2p1a[:], d["w2p1mc1"][:])
        w2p1b = wpool.tile([64, 4 * 128], BF16)
        nc.sync.dma_start(w2p1b[:], d["w2p1mc2"][:])
        gw = wpool.tile([128, 25 * NE], BF16)
        nc.sync.dma_start(gw[:], d["gwp"][:])
        e2w = wpool.tile([128, NE * DH], BF16)
        nc.sync.dma_start(e2w[:], d["e2w"][:])
        e1wt = []
        for e in range(NE):
            t = wpool.tile([128, 25 * DH], BF16, tag=f"e1w{e}", name=f"e1w{e}")
            nc.sync.dma_start(t[:], d["e1wr"][e])
            e1wt.append(t)
        b1c = wpool.tile([128, 1], F32); nc.sync.dma_start(b1c[:], d["b1col"][:])
        b2c = wpool.tile([128, 1], F32); nc.sync.dma_start(b2c[:], d["b2col"][:])
        gbc = wpool.tile([NE, 1], F32); nc.sync.dma_start(gbc[:], d["gbcol"][:])
        e1bT = wpool.tile([128, NE], F32); nc.sync.dma_start(e1bT[:], d["e1bT"][:])
        e2bT = wpool.tile([128, NE], F32); nc.sync.dma_start(e2bT[:], d["e2bT"][:])
        smw = wpool.tile([128, 10], BF16); nc.sync.dma_start(smw[:], d["smw"][:])
        smb5 = wpool.tile([1, 50], BF16); nc.sync.dma_start(smb5[:], d["smb5"][:])
        ident = wpool.tile([128, 128], F32)
        make_identity(nc, ident[:])
        ones = wpool.tile([1, 128], BF16)
        nc.scalar.activation(ones[:], e2w[0:1, 0:128], AF.Copy, scale=0.0, bias=1.0)

        loop_cm = tc.For_i(0, loop_reps, 1) if loop_reps else contextlib.nullcontext()
        with loop_cm:
         for ch in [c % NCH for c in range(NCH * body_mult)]:
            b0 = ch * NB
            tA0 = xtp.tile([100, 12, NB], BF16, tag="tA0", name="tA0")
            nc.sync.dma_start(tA0[:], d["xa"][:, ch, 0:12])
            tB0 = xtp.tile([60, 12, NB], BF16, tag="tB0", name="tB0")
            nc.sync.dma_start(tB0[:], d["xb"][:, ch, 0:12])
            tA1 = xtp.tile([100, 12, NB], BF16, tag="tA1", name="tA1")
            nc.sync.dma_start(tA1[:], d["xa"][:, ch, 12:24])
            tB1 = xtp.tile([60, 12, NB], BF16, tag="tB1", name="tB1")
            nc.sync.dma_start(tB1[:], d["xb"][:, ch, 12:24])

            pp0, pp1, htiles = [], [], []

            def conv2_block(P):
                y0 = 2 * P
                for yy in (y0, y0 + 1):
                    for mci in range(2):
                        ps = c2ps.tile([128, NB], F32, tag="ps", name="c2t")
                        for r in range(3):
                            nc.tensor.matmul(
                                ps[:], w2p0[:, 256 * r + 128 * mci:256 * r + 128 * mci + 128],
                                pp0[yy + r][:], start=(r == 0),
                                stop=(mci == 0 and r == 2))
                        if mci == 1:
                            for r in range(3):
                                nc.tensor.matmul(ps[:], w2p1a[:, 128 * r:128 * r + 128],
                                                 pp1[yy + r][:], start=False,
                                                 stop=(r == 2))
                        h = hpool.tile([128, NB], BF16, tag="h", name="ht")
                        nc.scalar.activation(h[:], ps[:], AF.Relu, bias=b2c[:, 0:1])
                        htiles.append(h)
                ps = c2ps.tile([128, NB], F32, tag="ps", name="c2t")
                for rr in range(4):
                    nc.tensor.matmul(ps[:], w2p1b[:, 128 * rr:128 * rr + 128],
                                     pp1[y0 + rr][:], start=(rr == 0), stop=(rr == 3))
                h = hpool.tile([128, NB], BF16, tag="h", name="ht")
                nc.scalar.activation(h[:], ps[:], AF.Relu, bias=b2c[:, 0:1])
                htiles.append(h)

            # ---- conv1 (one matmul per tile) + pool, conv2 interleaved ----
            for Y in range(12):
                y0, y1 = 2 * Y, 2 * Y + 1
                tA = tA0 if y0 < 12 else tA1
                tB = tB0 if y0 < 12 else tB1
                yo0, yo1 = y0 % 12, y1 % 12
                tms = []
                for wsl, dat, tg in ((w1a[:, 0:128], tA, "tm0"),
                                     (w1a[:, 128:256], tA, "tm2"),
                                     (w1b[:, 0:128], tB, "tm1")):
                    pa = c1ps.tile([128, NB], F32, tag="ps", name="c1t")
                    nc.tensor.matmul(pa[:], wsl, dat[:, yo0, :], start=True, stop=True)
                    pb = c1ps.tile([128, NB], F32, tag="ps", name="c1t")
                    nc.tensor.matmul(pb[:], wsl, dat[:, yo1, :], start=True, stop=True)
                    ra = tmp.tile([128, NB], BF16, tag=tg + "r", name="rat")
                    nc.scalar.activation(ra[:], pa[:], AF.Copy)
                    tm = tmp.tile([128, NB], BF16, tag=tg, name="tmt")
                    nc.vector.tensor_tensor(tm[:], pb[:], ra[:], op=ALU.max)
                    tms.append(tm)
                tm0, tm2, tm1 = tms
                p0 = tmp.tile([128, NB], BF16, tag="p0", name="p0t")
                nc.vector.tensor_tensor(p0[:], tm0[:], tm2[:], op=ALU.max)
                sh = shp.tile([64, NB], BF16, tag="sh", name="sht")
                nc.sync.dma_start(sh[:], tm1[64:128, :])
                p1 = tmp.tile([64, NB], BF16, tag="p1", name="p1t")
                nc.vector.tensor_tensor(p1[:], tm1[0:64, :], sh[:], op=ALU.max)
                t0 = pp0p.tile([128, NB], BF16, tag="pp0", name="pp0t")
                nc.vector.tensor_scalar(t0[:], p0[:], b1c[:, 0:1], 0.0,
                                        op0=ALU.add, op1=ALU.max)
                t1 = pp1p.tile([64, NB], BF16, tag="pp1", name="pp1t")
                nc.vector.tensor_scalar(t1[:], p1[:], b1c[0:64, 0:1], 0.0,
                                        op0=ALU.add, op1=ALU.max)
                pp0.append(t0)
                pp1.append(t1)
                if Y >= 3 and Y % 2 == 1:
                    conv2_block((Y - 3) // 2)

            # ---- gate ----
            gps = mps.tile([NE, NB], F32, tag="m", name="gpst")
            for kc in range(25):
                nc.tensor.matmul(gps[:], gw[:, NE * kc:NE * kc + NE], htiles[kc][:],
                                 start=(kc == 0), stop=(kc == 24))
            gsb = gp.tile([NE, NB], F32, tag="gsb")
            nc.scalar.activation(gsb[:], gps[:], AF.Identity, bias=gbc[:, 0:1])

            # ---- experts (h2 of expert e-1 interleaved under e's h1) ----
            h1s = [None] * NE
            h2t = [None] * NE

            def h2_for(e):
                h2ps = mps.tile([128, NB], F32, tag="m", name="h2pst")
                nc.tensor.matmul(h2ps[:], e2w[:, DH * e:DH * e + DH], h1s[e][:],
                                 start=True, stop=True)
                h2 = h2p.tile([128, NB], BF16, tag="h2", name="h2t_")
                nc.scalar.activation(h2[:], h2ps[:], AF.Tanh, bias=e2bT[:, e:e + 1])
                h2t[e] = h2

            for e in range(NE):
                h1ps = mps.tile([128, NB], F32, tag="m", name="h1pst")
                for kc in range(25):
                    nc.tensor.matmul(h1ps[:], e1wt[e][:, DH * kc:DH * kc + DH],
                                     htiles[kc][:], start=(kc == 0), stop=(kc == 24))
                h1 = h1p.tile([128, NB], BF16, tag="h1", name="h1t_")
                nc.scalar.activation(h1[:], h1ps[:], AF.Tanh, bias=e1bT[:, e:e + 1])
                h1s[e] = h1
                if e >= 1:
                    h2_for(e - 1)
            h2_for(NE - 1)

            # ---- heads: batched over the 4 token groups of the chunk ----
            NT = NB // 128
            hd4 = mps.tile([128, 6 * NT], F32, tag="m", name="hd4")
            for t4 in range(NT):
                tok = slice(128 * t4, 128 * t4 + 128)
                nc.tensor.transpose(hd4[:, 6 * t4:6 * t4 + NE], gsb[:, tok],
                                    ident[0:NE, 0:NE])
            s4 = smallp.tile([128, NT * NE], F32, tag="s4")
            nc.scalar.activation(
                s4[:].rearrange("p (t e) -> p t e", t=NT),
                hd4[:].rearrange("p (t x) -> p t x", t=NT)[:, :, 0:NE], AF.Copy)
            mx4 = smallp.tile([128, NT], F32, tag="mx4")
            nc.vector.reduce_max(mx4[:], s4[:].rearrange("p (t e) -> p t e", t=NT),
                                 axis=AX.X)
            z4 = smallp.tile([128, NT * NE], F32, tag="z4")
            nc.vector.tensor_tensor(
                z4[:].rearrange("p (t e) -> p t e", t=NT),
                s4[:].rearrange("p (t e) -> p t e", t=NT),
                mx4[:].unsqueeze(2).broadcast_to([128, NT, NE]), op=ALU.subtract)
            ex4 = smallp.tile([128, NT * NE], F32, tag="ex4")
            nc.scalar.activation(ex4[:], z4[:], AF.Exp)
            ex4v = ex4[:].rearrange("p (t i) -> p t i", t=NT)
            gt4 = smallp.tile([128, NT * NE * NE], F32, tag="gt4")
            a_b = ex4v.unsqueeze(2).broadcast_to([128, NT, NE, NE])
            b_b = ex4v.unsqueeze(3).broadcast_to([128, NT, NE, NE])
            nc.vector.tensor_tensor(
                gt4[:].rearrange("p (t i j) -> p t i j", t=NT, i=NE),
                a_b, b_b, op=ALU.is_gt)
            rank4 = smallp.tile([128, NT * NE], F32, tag="rank4")
            nc.vector.reduce_sum(
                rank4[:].rearrange("p (t i) -> p t i", t=NT),
                gt4[:].rearrange("p (t i j) -> p t i j", t=NT, i=NE), axis=AX.X)
            m014 = smallp.tile([128, NT * NE], F32, tag="m014")
            nc.vector.tensor_scalar(m014[:], rank4[:], float(TOPK) - 0.5, None,
                                    op0=ALU.is_le)
            wun4 = smallp.tile([128, NT * NE], F32, tag="wun4")
            nc.vector.tensor_mul(wun4[:], ex4[:], m014[:])
            ssum4 = smallp.tile([128, NT], F32, tag="ssum4")
            nc.vector.reduce_sum(ssum4[:], wun4[:].rearrange("p (t e) -> p t e", t=NT),
                                 axis=AX.X)
            rinv4 = smallp.tile([128, NT], F32, tag="rinv4")
            nc.vector.reciprocal(rinv4[:], ssum4[:])
            wfin4 = smallp.tile([128, NT * NE], F32, tag="wfin4")
            nc.vector.tensor_tensor(
                wfin4[:].rearrange("p (t e) -> p t e", t=NT),
                wun4[:].rearrange("p (t e) -> p t e", t=NT),
                rinv4[:].unsqueeze(2).broadcast_to([128, NT, NE]), op=ALU.mult)

            lg4 = smallp.tile([128, NT * 10], F32, tag="lg4")
            for t4 in range(NT):
                tok = slice(128 * t4, 128 * t4 + 128)
                lep = mps.tile([128, 50], F32, tag="m", name="lept")
                nc.tensor.matmul(lep[:, 0:50], ones[:], smb5[:], start=True, stop=False)
                for e in range(NE):
                    nc.tensor.matmul(lep[:, 10 * e:10 * e + 10], h2t[e][:, tok],
                                     smw[:], start=False, stop=(e == NE - 1))
                scl = smallp.tile([128, 50], F32, tag="scl")
                nc.vector.tensor_tensor(
                    scl[:].rearrange("p (e k) -> p e k", e=NE),
                    lep[:].rearrange("p (e k) -> p e k", e=NE),
                    wfin4[:, NE * t4:NE * t4 + NE].unsqueeze(2).broadcast_to(
                        [128, NE, 10]), op=ALU.mult)
                nc.vector.reduce_sum(lg4[:, 10 * t4:10 * t4 + 10],
                                     scl[:].rearrange("p (e k) -> p k e", e=NE),
                                     axis=AX.X)
            lmx4 = smallp.tile([128, NT], F32, tag="lmx4")
            nc.vector.reduce_max(lmx4[:], lg4[:].rearrange("p (t k) -> p t k", t=NT),
                                 axis=AX.X)
            zz4 = smallp.tile([128, NT * 10], F32, tag="zz4")
            nc.vector.tensor_tensor(
                zz4[:].rearrange("p (t k) -> p t k", t=NT),
                lg4[:].rearrange("p (t k) -> p t k", t=NT),
                lmx4[:].unsqueeze(2).broadcast_to([128, NT, 10]), op=ALU.subtract)
            lex4 = smallp.tile([128, NT * 10], F32, tag="lex4")
            nc.scalar.activation(lex4[:], zz4[:], AF.Exp)
            lsum4 = smallp.tile([128, NT], F32, tag="lsum4")
            nc.vector.reduce_sum(lsum4[:], lex4[:].rearrange("p (t k) -> p t k", t=NT),
                                 axis=AX.X)
            lrinv4 = smallp.tile([128, NT], F32, tag="lrinv4")
            nc.vector.reciprocal(lrinv4[:], lsum4[:])
            prob4 = smallp.tile([128, NT * 10], F32, tag="prob4")
            nc.vector.tensor_tensor(
                prob4[:].rearrange("p (t k) -> p t k", t=NT),
                lex4[:].rearrange("p (t k) -> p t k", t=NT),
                lrinv4[:].unsqueeze(2).broadcast_to([128, NT, 10]), op=ALU.mult)
            nc.sync.dma_start(
                out_d[b0:b0 + NB, :].rearrange("(t p) k -> p t k", t=NT),
                prob4[:].rearrange("p (t k) -> p t k", t=NT))


_NC_CACHE = None


def _get_nc():
    global _NC_CACHE
    if _NC_CACHE is None:
        _NC_CACHE = _build_nc()
    return _NC_CACHE


def kernel(**inputs):
    percore, w = _host_prep(**inputs)
    in_maps = [{**percore[c], **w} for c in range(NCORES)]
    nc = _get_nc()
    trace = bool(int(os.environ.get("KERNEL_TRACE", "0")))
    res = run_bass_kernel_spmd(nc, in_maps, list(range(NCORES)), trace=trace)
    kernel.last_results = res
    out = np.concatenate([res.results[c]["out"] for c in range(NCORES)], axis=0)
    return out.astype(np.float32)
